# revision 1
# baseline (speedup 1.0000x reference)
"""Multi-head attention kernel for Trainium2, 8 NeuronCores.

Problem: B=2, S=4096, D=512, H=8 heads (dk=64), explicit S x S masked softmax.

Sharding: batch (2) x query-row-blocks (4) -> 8 cores. Each core computes all 8
heads for 1024 query rows of one batch element. K/V projections are computed
per-core for the full sequence (duplicated across the 4 cores of a batch).

Per-core layout choices:
  - scores computed transposed ([keys, q]) so the PV matmul consumes them
    directly (no on-chip transposes anywhere).
  - mask is host-transposed to [S, QR] and DMA-cast int32->bf16 on load.
  - softmax sums come from a ones-column appended to V (M=65 PV matmul);
    the reciprocal row is broadcast across partitions via a DRAM bounce
    (stride-0 partition DMA) and the normalize-multiply of each head is
    deferred into the next head's pipeline so unit boundaries never stall.
  - all matmul operands bf16 (DMA-cast on load), fp32 accumulation in PSUM.
  - emission is a flat software pipeline over (head, group) with score
    matmuls issued two blocks ahead; output-projection chunks are spread
    across subsequent heads. ScalarE (exp) runs at ~97-100%% occupancy.
"""

import numpy as np

B, S, D, H = 2, 4096, 512, 8
DK = D // H            # 64
NCORES = 8
RG = 4                 # row groups per batch
QR = S // RG           # 1024 query rows per core
QT = 512               # query tile
NQT = QR // QT         # 2
KBS = 128              # key block size
KB = S // KBS          # 32 key blocks
G = 3                  # key blocks per ACT exp group (3 PSUM banks, FD=1536)

_BUILT = None


def _build():
    import concourse.bacc as bacc
    import concourse.mybir as mybir
    import concourse.tile as tile
    from concourse.bass_interp import get_hw_module

    F32 = mybir.dt.float32
    BF16 = mybir.dt.bfloat16
    I32 = mybir.dt.int32
    EXP = mybir.ActivationFunctionType.Exp
    MULT = mybir.AluOpType.mult

    nc = bacc.Bacc("TRN2", target_bir_lowering=False, debug=False,
                   enable_asserts=False, num_devices=NCORES)

    qT = nc.dram_tensor("qT", [D, QR], F32, kind="ExternalInput")
    kT = nc.dram_tensor("kT", [D, S], F32, kind="ExternalInput")
    vT = nc.dram_tensor("vT", [D, S], F32, kind="ExternalInput")
    maskT = nc.dram_tensor("maskT", [S, QR], I32, kind="ExternalInput")
    wq = nc.dram_tensor("wq", [D, D], F32, kind="ExternalInput")
    wk = nc.dram_tensor("wk", [D, D], F32, kind="ExternalInput")
    wv = nc.dram_tensor("wv", [D, D], F32, kind="ExternalInput")
    wo = nc.dram_tensor("wo", [D, D], F32, kind="ExternalInput")
    out = nc.dram_tensor("out", [QR, D], F32, kind="ExternalOutput")
    # DRAM bounce buffer for broadcasting softmax reciprocals across partitions
    rcd = nc.dram_tensor("rcd", [NQT * H, 512], F32, kind="Internal")

    with tile.TileContext(nc) as tc:
        with tc.tile_pool(name="persist", bufs=1) as persist, \
             tc.tile_pool(name="maskp", bufs=1) as maskp:

            # persistent tiles
            KT = persist.tile([128, 4, S], BF16)      # K^T, 4 d_out chunks
            QTt = persist.tile([128, 4, QR], BF16)    # Q^T
            VA = persist.tile([128, KB, H * 65], BF16)  # V + ones col per head
            maskA = maskp.tile([128, KB, QT], BF16)
            ones_t = persist.tile([128, 64], F32)
            nc.vector.memset(ones_t, 1.0)

            # ones column of VA (head-interleaved: col h*65+64)
            va_ones = VA.rearrange("p kb (h x) -> p kb h x", x=65)[:, :, :, 64:65]
            nc.gpsimd.memset(va_ones, 1.0)

            mask_src = maskT[:, :].rearrange("(kb p) q -> p kb q", p=128)

            # ---------------- projections ----------------
            # Order: V first, then K(dc0) + Q(first tile), then two "warmup"
            # attention heads with small exp groups run during the remaining
            # projection work (ACT is otherwise idle there), then the rest.
            HS = S // 2  # 2048
            NW = 0       # warmup units
            units = [(qt, h) for qt in range(NQT) for h in range(H)]
            xts = {}

            with tc.tile_pool(name="pxt", bufs=2) as pxt, \
                 tc.tile_pool(name="pwrk", bufs=3) as pwrk:

                import concourse.bass as bass

                def emit_norm1(ui, pv):
                    # reciprocal of the sums row, bounced through DRAM to
                    # broadcast it across partitions 0-63 (SBUF APs cannot
                    # have a zero partition stride; DRAM APs can)
                    rc = pwrk.tile([128, QT], F32, tag="rc", name=f"rc{ui}")
                    nc.vector.reciprocal(rc[64:65, :], pv[64:65, :])
                    nc.sync.dma_start(out=rcd[ui:ui + 1, :], in_=rc[64:65, :])
                    bcs = pwrk.tile([64, QT], F32, tag="bcs", name=f"bcs{ui}")
                    src = rcd[ui:ui + 1, :]
                    bsrc = bass.AP(tensor=src.tensor, offset=src.offset,
                                   ap=[[0, 64]] + [list(a) for a in src.ap[1:]])
                    nc.sync.dma_start(out=bcs, in_=bsrc)
                    return bcs

                def emit_norm2(qt, h, pv, bcs):
                    nc.vector.tensor_tensor(xts[qt][:, h, :], pv[0:64, :],
                                            bcs, op=MULT)

                with tc.tile_pool(name="pin", bufs=1) as pin, \
                     tc.tile_pool(name="pint", bufs=2) as pint, \
                     tc.tile_pool(name="pexw", bufs=3) as pexw, \
                     tc.tile_pool(name="pps", bufs=4, space="PSUM") as pps, \
                     tc.tile_pool(name="pscw", bufs=2, space="PSUM") as pscw, \
                     tc.tile_pool(name="ppvw", bufs=1, space="PSUM") as ppvw:

                    pscw._bctag = "scw"
                    wk_bf = pin.tile([128, 4, D], BF16, tag="wk")
                    wq_bf = pin.tile([128, 4, D], BF16, tag="wq")
                    wv_bf = pin.tile([128, 4, D], BF16, tag="wv")
                    qtin = pin.tile([128, 4, QR], BF16, tag="qtin")
                    kT_src = kT[:, :].rearrange("(c p) s -> p c s", p=128)
                    vT_src = vT[:, :].rearrange("(c p) s -> p c s", p=128)

                    nc.gpsimd.dma_start(
                        out=wk_bf,
                        in_=wk[:, :].rearrange("(c p) d -> p c d", p=128))

                    # K^T projection, half-slab major
                    for hf in range(2):
                        ktin = pint.tile([128, 4, HS], BF16, tag="tin",
                                         name=f"ktin{hf}")
                        for qh in range(2):
                            a = hf * HS + qh * (HS // 2)
                            nc.gpsimd.dma_start(
                                out=ktin[:, :, qh * (HS // 2):
                                         (qh + 1) * (HS // 2)],
                                in_=kT_src[:, :, a:a + HS // 2])
                        if hf == 1:
                            nc.gpsimd.dma_start(
                                out=wq_bf,
                                in_=wq[:, :].rearrange("(c p) d -> p c d",
                                                       p=128))
                            nc.gpsimd.dma_start(
                                out=wv_bf,
                                in_=wv[:, :].rearrange("(c p) d -> p c d",
                                                       p=128))
                            nc.gpsimd.dma_start(
                                out=qtin,
                                in_=qT[:, :].rearrange("(c p) r -> p c r",
                                                       p=128))
                        for dc in range(4):
                            for ns in range(HS // 512):
                                s0 = hf * HS + ns * 512
                                pt = pps.tile([128, 512], F32, tag="pt",
                                              name=f"ptk{dc}_{hf}_{ns}")
                                for di in range(4):
                                    nc.tensor.matmul(
                                        pt,
                                        wk_bf[:, di, dc * 128:(dc + 1) * 128],
                                        ktin[:, di,
                                             ns * 512:(ns + 1) * 512],
                                        start=(di == 0), stop=(di == 3))
                                nc.vector.tensor_copy(
                                    KT[:, dc, s0:s0 + 512], pt)

                    # Q^T projection
                    for dc in range(4):
                        for ns in range(QR // 512):
                            pt = pps.tile([128, 512], F32, tag="pt",
                                          name=f"ptq{dc}_{ns}")
                            for di in range(4):
                                nc.tensor.matmul(
                                    pt,
                                    wq_bf[:, di, dc * 128:(dc + 1) * 128],
                                    qtin[:, di, ns * 512:(ns + 1) * 512],
                                    start=(di == 0), stop=(di == 3))
                            nc.vector.tensor_copy(
                                QTt[:, dc, ns * 512:(ns + 1) * 512], pt)

                    nc.gpsimd.dma_start(out=maskA[:, 0:8, :],
                                        in_=mask_src[:, 0:8, 0:QT])

                    # V projection, half-slab major, scattered into VA
                    for hf in range(2):
                        vtin = pint.tile([128, 4, HS], BF16, tag="tin",
                                         name=f"vtin{hf}")
                        for qh in range(2):
                            a = hf * HS + qh * (HS // 2)
                            nc.gpsimd.dma_start(
                                out=vtin[:, :, qh * (HS // 2):
                                         (qh + 1) * (HS // 2)],
                                in_=vT_src[:, :, a:a + HS // 2])
                        for si in range(HS // 128):
                            sc_i = hf * (HS // 128) + si
                            pt = pps.tile([128, 512], F32, tag="pt",
                                          name=f"ptv{sc_i}")
                            for di in range(4):
                                nc.tensor.matmul(
                                    pt,
                                    vtin[:, di, si * 128:(si + 1) * 128],
                                    wv_bf[:, di, :],
                                    start=(di == 0), stop=(di == 3))
                            dst = VA[:, sc_i, :].rearrange(
                                "p (h x) -> p h x", x=65)[:, :, 0:64]
                            nc.vector.tensor_copy(
                                dst, pt.rearrange("p (h x) -> p h x", x=64))

                    nc.gpsimd.dma_start(out=maskA[:, 8:16, :],
                                        in_=mask_src[:, 8:16, 0:QT])
                    nc.gpsimd.dma_start(out=maskA[:, 16:24, :],
                                        in_=mask_src[:, 16:24, 0:QT])
                    nc.gpsimd.dma_start(out=maskA[:, 24:32, :],
                                        in_=mask_src[:, 24:32, 0:QT])

                # ---------------- main attention ----------------
                with tc.tile_pool(name="late", bufs=1) as late, \
                     tc.tile_pool(name="pex", bufs=4) as pex, \
                     tc.tile_pool(name="psc", bufs=2, space="PSUM") as psc, \
                     tc.tile_pool(name="ppv", bufs=2, space="PSUM") as ppv:

                    ppv._bctag = "pvb"
                    WO64 = late.tile([64, H, D], BF16)
                    nc.gpsimd.dma_start(
                        out=WO64,
                        in_=wo[:, :].rearrange("(h p) d -> p h d", p=64))
                    maskB = late.tile([128, KB, QT], BF16)
                    for mc in range(4):
                        nc.gpsimd.dma_start(
                            out=maskB[:, mc * 8:(mc + 1) * 8, :],
                            in_=mask_src[:, mc * 8:(mc + 1) * 8, QT:QR])

                    groups = []
                    kb0 = 0
                    while kb0 < KB:
                        groups.append((kb0, min(G, KB - kb0)))
                        kb0 += G

                    def emit_scores(qt, h, kb0, gn):
                        pb = (h % 2) * 64
                        hc = h // 2
                        sc = psc.tile([128, G, QT], F32, tag="sc")
                        for i in range(gn):
                            kb = kb0 + i
                            nc.tensor.matmul(
                                sc[:, i, :],
                                KT[pb:pb + 64, hc, kb * 128:(kb + 1) * 128],
                                QTt[pb:pb + 64, hc, qt * QT:(qt + 1) * QT],
                                start=True, stop=True)
                        return sc

                    pending_wo = []

                    def emit_wo(qt, qc):
                        xt = xts[qt]
                        po = ppv.tile([128, D], F32, tag="pvb",
                                      name=f"po{qt}_{qc}")
                        for hh in range(H):
                            nc.tensor.matmul(
                                po,
                                xt[:, hh, qc * 128:(qc + 1) * 128],
                                WO64[:, hh, :],
                                start=(hh == 0), stop=(hh == H - 1))
                        outt = late.tile([128, D], F32, tag="outt", bufs=3)
                        nc.vector.tensor_copy(outt, po)
                        nc.sync.dma_start(
                            out=out[qt * QT + qc * 128:
                                    qt * QT + (qc + 1) * 128, :],
                            in_=outt)

                    # flat software pipeline over (unit, group); scores are
                    # emitted two blocks ahead so ACT never waits on the PE
                    flat = []
                    for ui in range(NW, len(units)):
                        qt, h = units[ui]
                        for gi, (kb0, gn) in enumerate(groups):
                            flat.append((ui, qt, h, gi, kb0, gn))

                    sc_tiles = {}

                    def emit_sc(idx):
                        _, qt, h, _, kb0, gn = flat[idx]
                        sc_tiles[idx] = emit_scores(qt, h, kb0, gn)

                    emit_sc(0)
                    emit_sc(1)
                    pv = None
                    pending_norm = None
                    wo_inflight = None
                    for idx, (ui, qt, h, gi, kb0, gn) in enumerate(flat):
                        if h == 0 and gi == 0 and qt not in xts:
                            xts[qt] = pxt.tile([64, H, QT], BF16, tag="xt",
                                               name=f"xt{qt}")
                        xt = xts[qt]
                        if gi == 0:
                            pv = ppv.tile([128, QT], F32, tag="pvb",
                                          name=f"pv{ui}")
                        if idx + 2 < len(flat):
                            emit_sc(idx + 2)
                        sc = sc_tiles.pop(idx)
                        ex = pex.tile([128, G, QT], BF16, tag="ex")
                        nc.scalar.activation(ex[:, 0:gn, :], sc[:, 0:gn, :],
                                             EXP, scale=0.125)
                        mk = pex.tile([128, G, QT], BF16, tag="mk")
                        nc.vector.tensor_tensor(
                            mk[:, 0:gn, :], ex[:, 0:gn, :],
                            (maskA if qt == 0 else maskB)[:, kb0:kb0 + gn, :],
                            op=MULT)
                        for i in range(gn):
                            kb = kb0 + i
                            nc.tensor.matmul(
                                pv[0:65, :],
                                VA[:, kb, h * 65:(h + 1) * 65],
                                mk[:, i, :],
                                start=(kb == 0), stop=(kb == KB - 1))
                        if wo_inflight is not None and gi == 7:
                            qtw, qcw, po = wo_inflight
                            wo_inflight = None
                            for hh in range(4, H):
                                nc.tensor.matmul(
                                    po,
                                    xts[qtw][:, hh, qcw * 128:(qcw + 1) * 128],
                                    WO64[:, hh, :],
                                    start=False, stop=(hh == H - 1))
                            outt = late.tile([128, D], F32, tag="outt",
                                             bufs=3)
                            nc.vector.tensor_copy(outt, po)
                            nc.sync.dma_start(
                                out=out[qtw * QT + qcw * 128:
                                        qtw * QT + (qcw + 1) * 128, :],
                                in_=outt)
                        if gi == 2 and pending_norm is not None:
                            # deferred normalize-multiply of the previous
                            # unit (its pv slot frees here, mid-unit, so the
                            # boundary never serializes on the norm chain)
                            emit_norm2(*pending_norm)
                            pending_norm = None
                        elif gi == 6 and pending_wo:
                            qtw, qcw = pending_wo.pop(0)
                            po = ppv.tile([128, D], F32, tag="pvb",
                                          name=f"po{qtw}_{qcw}")
                            for hh in range(4):
                                nc.tensor.matmul(
                                    po,
                                    xts[qtw][:, hh, qcw * 128:(qcw + 1) * 128],
                                    WO64[:, hh, :],
                                    start=(hh == 0), stop=False)
                            wo_inflight = (qtw, qcw, po)
                        if gi == len(groups) - 1:
                            if ui == len(units) - 1:
                                # final unit: fast-path norm via PE broadcast
                                # (score PSUM slots are free at this point)
                                rc = pwrk.tile([128, QT], F32, tag="rc",
                                               name="rcF")
                                nc.vector.reciprocal(rc[64:65, :],
                                                     pv[64:65, :])
                                bct = psc.tile([128, G, QT], F32, tag="sc",
                                               name="bcF")
                                nc.tensor.matmul(bct[0:64, 0, :],
                                                 ones_t[64:65, :],
                                                 rc[64:65, :],
                                                 start=True, stop=True)
                                bcs = pwrk.tile([64, QT], F32, tag="bcs",
                                                name="bcsF")
                                nc.vector.tensor_copy(bcs, bct[0:64, 0, :])
                                emit_norm2(qt, h, pv, bcs)
                                pending_wo.extend(
                                    (qt, qc) for qc in range(QT // 128))
                                while pending_wo:
                                    emit_wo(*pending_wo.pop(0))
                            else:
                                bcs = emit_norm1(ui, pv)
                                pending_norm = (qt, h, pv, bcs)
                                if h == H - 1:
                                    pending_wo.extend(
                                        (qt, qc) for qc in range(QT // 128))

    nc.compile()
    nc.m = get_hw_module(nc.m)
    return nc


def _get_built():
    global _BUILT
    if _BUILT is None:
        _BUILT = _build()
    return _BUILT


def kernel(q, k, v, mask, w_q, w_k, w_v, w_o):
    import os
    # NTFF tracing needs antenv.axon_hooks, absent in some environments;
    # never let an inherited BASS_TRACE env var route us into that path.
    os.environ.setdefault("BASS_NEVER_TRACE", "1")
    from concourse.bass_utils import run_bass_kernel_spmd

    q = np.asarray(q, dtype=np.float32)
    k = np.asarray(k, dtype=np.float32)
    v = np.asarray(v, dtype=np.float32)
    mask = np.asarray(mask, dtype=np.int32)
    w_q = np.asarray(w_q, dtype=np.float32)
    w_k = np.asarray(w_k, dtype=np.float32)
    w_v = np.asarray(w_v, dtype=np.float32)
    w_o = np.asarray(w_o, dtype=np.float32)

    nc = _get_built()

    kT = [np.ascontiguousarray(k[b].T) for b in range(B)]
    vT = [np.ascontiguousarray(v[b].T) for b in range(B)]
    maskT = [np.ascontiguousarray(mask[b].T) for b in range(B)]

    in_maps = []
    for c in range(NCORES):
        b, r = divmod(c, RG)
        q0 = r * QR
        in_maps.append({
            "qT": np.ascontiguousarray(q[b, q0:q0 + QR, :].T),
            "kT": kT[b],
            "vT": vT[b],
            "maskT": np.ascontiguousarray(maskT[b][:, q0:q0 + QR]),
            "wq": w_q, "wk": w_k, "wv": w_v, "wo": w_o,
        })

    global _LAST_IN_MAPS
    _LAST_IN_MAPS = in_maps
    res = run_bass_kernel_spmd(nc, in_maps, list(range(NCORES)))

    full = np.empty((B, S, D), dtype=np.float32)
    for c in range(NCORES):
        b, r = divmod(c, RG)
        full[b, r * QR:(r + 1) * QR, :] = res.results[c]["out"]
    return full



# revision 15
# speedup vs baseline: 1.0945x; 1.0945x over previous
"""Multi-head attention kernel for Trainium2, 8 NeuronCores.

Problem: B=2, S=4096, D=512, H=8 heads (dk=64), explicit S x S masked softmax.

Sharding (Megatron-style tensor parallel): batch (2) x head-pairs (4) -> 8
cores. Each core computes 2 heads for ALL 4096 queries of one batch element,
with w_q/w_k/w_v column-sliced (128 cols) and w_o row-sliced (128 rows).
Each core emits a PARTIAL [S, D] output (its heads' w_o contribution); the
host sums the 4 partials per batch during unsharding. This removes the 4x
K/V projection duplication of a query-sharded layout: per-core projection
work drops from 180K to 49K PE cycles, so the serial projection prologue
shrinks ~4x and the steady state is ACT(exp)-limited.

Per-core layout choices:
  - scores computed transposed ([keys, q]) so the PV matmul consumes them
    directly (no on-chip transposes anywhere).
  - mask streamed per query-tile as uint8 and DMA-cast to bf16 on load.
  - softmax sums come from a ones-column appended to V (M=65 PV matmul);
    the reciprocal row is broadcast across partitions via a DRAM bounce
    (stride-0 partition DMA); normalize-multiplies are deferred into the
    next unit's pipeline so unit boundaries never stall.
  - both heads' normalized outputs are packed into one [128, QT] tile
    (DVE writes head 1 at partitions 64-127), so the output projection is
    a single 128-contraction matmul per 128 query rows.
  - ALL psum flows through two pools (scores ring 2x3 banks + pv/wo ring
    2x1): projections borrow score-ring slices, so leftover projection
    work (V quarters, Q tiles) drains into early attention groups where
    the PE has slack against the ACT exp rate.
  - all matmul operands bf16 (host- or DMA-cast), fp32 accumulation in PSUM.
"""

import numpy as np

B, S, D, H = 2, 4096, 512, 8
DK = D // H            # 64
NCORES = 8
HP = 2                 # heads per core
NQT = 8                # query tiles per core
QT = 512               # query tile
KBS = 128              # key block size
KB = S // KBS          # 32 key blocks
G = 3                  # key blocks per ACT exp group (3 PSUM banks, FD=1536)
QTR = 1024             # input staging quarter (columns)

_BUILT = None


def _build():
    import concourse.bacc as bacc
    import concourse.mybir as mybir
    import concourse.tile as tile
    from concourse.bass_interp import get_hw_module

    F32 = mybir.dt.float32
    BF16 = mybir.dt.bfloat16
    U8 = mybir.dt.uint8
    EXP = mybir.ActivationFunctionType.Exp
    MULT = mybir.AluOpType.mult

    nc = bacc.Bacc("TRN2", target_bir_lowering=False, debug=False,
                   enable_asserts=False, num_devices=NCORES)

    qT = nc.dram_tensor("qT", [D, S], BF16, kind="ExternalInput")
    kT = nc.dram_tensor("kT", [D, S], BF16, kind="ExternalInput")
    vT = nc.dram_tensor("vT", [D, S], BF16, kind="ExternalInput")
    maskP = nc.dram_tensor("maskP", [NQT, 128, KB, QT], U8,
                           kind="ExternalInput")
    wq = nc.dram_tensor("wq", [D, HP * DK], BF16, kind="ExternalInput")
    wk = nc.dram_tensor("wk", [D, HP * DK], BF16, kind="ExternalInput")
    wv = nc.dram_tensor("wv", [D, HP * DK], BF16, kind="ExternalInput")
    wo = nc.dram_tensor("wo", [HP * DK, D], BF16, kind="ExternalInput")
    out = nc.dram_tensor("out", [S, D], F32, kind="ExternalOutput")
    # DRAM bounce buffer for broadcasting softmax reciprocals across partitions
    rcd = nc.dram_tensor("rcd", [NQT * HP, 512], F32, kind="Internal")

    import concourse.bass as bass

    with tile.TileContext(nc) as tc:
        with tc.tile_pool(name="persist", bufs=1) as persist, \
             tc.tile_pool(name="maskp", bufs=3) as maskp, \
             tc.tile_pool(name="pstg", bufs=3) as pstg, \
             tc.tile_pool(name="pxt", bufs=2) as pxt, \
             tc.tile_pool(name="pwrk", bufs=3) as pwrk, \
             tc.tile_pool(name="pex", bufs=4) as pex, \
             tc.tile_pool(name="late", bufs=1) as late, \
             tc.tile_pool(name="psc", bufs=2, space="PSUM") as psc, \
             tc.tile_pool(name="ppv", bufs=2, space="PSUM") as ppv:

            ppv._bctag = "pvb"

            # ---------------- persistent tiles ----------------
            KT = persist.tile([128, S], BF16)        # K^T (2 heads' dk rows)
            QTt = persist.tile([128, S], BF16)       # Q^T
            VA = persist.tile([128, KB, HP * 65], BF16)  # V + ones col/head
            ones_t = persist.tile([128, 64], F32)
            nc.vector.memset(ones_t, 1.0)
            WO128 = persist.tile([128, D], BF16)     # both heads' wo rows
            wk_b = persist.tile([128, 4, 128], BF16)
            wq_b = persist.tile([128, 4, 128], BF16)
            wv_b = persist.tile([128, 4, 128], BF16)

            va_ones = VA.rearrange("p kb (h x) -> p kb h x", x=65)[:, :, :, 64:65]
            nc.gpsimd.memset(va_ones, 1.0)

            kT_src = kT[:, :].rearrange("(c p) s -> p c s", p=128)
            vT_src = vT[:, :].rearrange("(c p) s -> p c s", p=128)
            qT_src = qT[:, :].rearrange("(c p) s -> p c s", p=128)

            maskq = {}

            def load_mask(qt, mc0=0, mc1=4):
                # 4 preemptible sub-DMAs so latency-sensitive transfers
                # (rcd bounce) interleave between them
                if qt in maskq:
                    mq = maskq[qt]
                else:
                    mq = maskp.tile([128, KB, QT], BF16, tag="mq",
                                    name=f"mq{qt}")
                    maskq[qt] = mq
                for mc in range(4 * mc0, 4 * mc1):
                    nc.gpsimd.dma_start(
                        out=mq[:, mc * 2:(mc + 1) * 2, :],
                        in_=maskP[qt, :, mc * 2:(mc + 1) * 2, :])

            # ---------------- projections (quarter-staged) ----------------
            # All proj psum borrows score-ring tiles ([128, 3, 512] = 3
            # banks), so projections and attention share one PSUM layout.
            def stage(src, qtr, name):
                t = pstg.tile([128, 4, QTR], BF16, tag="stg", name=name)
                nc.gpsimd.dma_start(
                    out=t, in_=src[:, :, qtr * QTR:(qtr + 1) * QTR])
                return t

            def proj_kq(w_b, tin, dst, toff, nst, name):
                # dst[:, toff : toff+nst*512] from one staged quarter
                pt = psc.tile([128, G, QT], F32, tag="sc", name=f"pp{name}")
                for ns in range(nst):
                    for di in range(4):
                        nc.tensor.matmul(
                            pt[:, ns, :], w_b[:, di, :],
                            tin[:, di, ns * 512:(ns + 1) * 512],
                            start=(di == 0), stop=(di == 3))
                nc.vector.tensor_copy(
                    dst[:, toff:toff + nst * 512],
                    pt[:, 0:nst, :].rearrange("p a b -> p (a b)"))

            def proj_v(tin, sc0):
                # 8 VA chunks [128 s-rows, 128 v-cols] from one staged quarter
                pt = psc.tile([128, G, QT], F32, tag="sc", name=f"ppv{sc0}")
                for i in range(8):
                    si = (sc0 + i) % 8
                    for di in range(4):
                        nc.tensor.matmul(
                            pt[:, i // 4, (i % 4) * 128:(i % 4 + 1) * 128],
                            tin[:, di, si * 128:(si + 1) * 128],
                            wv_b[:, di, :],
                            start=(di == 0), stop=(di == 3))
                src = pt.rearrange("p g (qc hx) -> p (g qc) hx", hx=128)
                src = src.rearrange("p c (h x) -> p c h x", x=64)[:, 0:8]
                dst = VA[:, sc0:sc0 + 8, :].rearrange(
                    "p kb (h x) -> p kb h x", x=65)[:, :, :, 0:64]
                nc.vector.tensor_copy(dst, src)

            # prologue DMA order is latency-critical: first-needed first.
            # Only K quarter 0 / Q quarter 0 / mask quarter 0 / V quarter 0
            # are projected before attention emission starts; everything
            # else drains into early attention groups in consumption order.
            kq0 = stage(kT_src, 0, "kq0")
            nc.gpsimd.dma_start(
                out=wk_b, in_=wk[:, :].rearrange("(c p) d -> p c d", p=128))
            qq0 = stage(qT_src, 0, "qq0")
            nc.gpsimd.dma_start(
                out=wq_b, in_=wq[:, :].rearrange("(c p) d -> p c d", p=128))
            load_mask(0, 0, 1)
            vq0 = stage(vT_src, 0, "vq0")
            nc.gpsimd.dma_start(
                out=wv_b, in_=wv[:, :].rearrange("(c p) d -> p c d", p=128))
            proj_kq(wk_b, kq0, KT, 0, 2, "k0")
            kq1 = stage(kT_src, 1, "kq1")
            proj_kq(wq_b, qq0, QTt, 0, 2, "q0")
            load_mask(0, 1, 2)
            vq1 = stage(vT_src, 1, "vq1")
            staged = {("kq", 1): kq1, ("v", 0): vq0, ("v", 1): vq1}

            # drained actions, keyed by (unit, group) emission slots; each
            # runs on the shared psc ring / DMA queues in consumption order
            def a_dma_kq(qtr):
                return lambda: staged.__setitem__(
                    ("kq", qtr), stage(kT_src, qtr, f"kq{qtr}"))

            def a_dma_vq(qtr):
                return lambda: staged.__setitem__(
                    ("v", qtr), stage(vT_src, qtr, f"vq{qtr}"))

            def a_dma_qq(qtr):
                return lambda: staged.__setitem__(
                    ("q", qtr), stage(qT_src, qtr, f"qq{qtr}"))

            def a_k(qtr):
                return lambda: proj_kq(wk_b, staged.pop(("kq", qtr)), KT,
                                       qtr * QTR, 2, f"k{qtr}")

            def a_v(qtr):
                return lambda: proj_v(staged.pop(("v", qtr)), qtr * 8)

            def a_q(qtr):
                return lambda: proj_kq(wq_b, staged.pop(("q", qtr)), QTt,
                                       qtr * QTR, 2, f"q{qtr}")

            def a_m0(quarter):
                return lambda: load_mask(0, quarter, quarter + 1)

            def a_m1(quarter):
                return lambda: load_mask(1, quarter, quarter + 1)

            def a_wo():
                return lambda: nc.gpsimd.dma_start(out=WO128, in_=wo[:, :])

            drain_slots = {
                (0, 0): [a_k(1), a_v(0), a_m0(2)],
                (0, 1): [a_dma_kq(2), a_dma_vq(2)],
                (0, 2): [a_v(1), a_m0(3)],
                (0, 3): [a_k(2)],
                (0, 5): [a_v(2), a_dma_kq(3), a_dma_vq(3)],
                (0, 6): [a_k(3)],
                (0, 7): [a_m1(0)],
                (0, 8): [a_v(3), a_m1(1)],
                (0, 9): [a_m1(2)],
                (0, 10): [a_m1(3), a_wo()],
                (1, 1): [a_dma_qq(1)],
                (2, 5): [a_q(1)],
                (2, 7): [a_dma_qq(2)],
                (3, 5): [a_q(2)],
                (3, 7): [a_dma_qq(3)],
                (4, 5): [a_q(3)],
            }

            def drain_task(ui, gi):
                for a in drain_slots.get((ui, gi), ()):
                    a()

            # ---------------- attention ----------------
            units = [(qt, h) for qt in range(NQT) for h in range(HP)]
            xts = {}

            def emit_norm1(ui, pv):
                # reciprocal of the sums row, bounced through DRAM to
                # broadcast it across partitions 0-63 (SBUF APs cannot
                # have a zero partition stride; DRAM APs can). The SP
                # queue carries only this + the small out-writes, so the
                # latency-sensitive bounce never queues behind bulk DMAs.
                rc = pwrk.tile([128, QT], F32, tag="rc", name=f"rc{ui}")
                nc.vector.reciprocal(rc[64:65, :], pv[64:65, :])
                nc.sync.dma_start(out=rcd[ui:ui + 1, :], in_=rc[64:65, :])
                bcs = pwrk.tile([64, QT], F32, tag="bcs", name=f"bcs{ui}")
                src = rcd[ui:ui + 1, :]
                bsrc = bass.AP(tensor=src.tensor, offset=src.offset,
                               ap=[[0, 64]] + [list(a) for a in src.ap[1:]])
                nc.sync.dma_start(out=bcs, in_=bsrc)
                return bcs

            def emit_norm2(qt, h, pv, bcs):
                # head h lands at partitions h*64 .. h*64+63 (packed xt)
                nc.vector.tensor_tensor(
                    xts[qt][h * 64:(h + 1) * 64, :], pv[0:64, :], bcs, op=MULT)

            groups = []
            kb0 = 0
            while kb0 < KB:
                groups.append((kb0, min(G, KB - kb0)))
                kb0 += G

            def emit_scores(qt, h, kb0, gn):
                pb = h * 64
                sc = psc.tile([128, G, QT], F32, tag="sc")
                for i in range(gn):
                    kb = kb0 + i
                    nc.tensor.matmul(
                        sc[:, i, :],
                        KT[pb:pb + 64, kb * 128:(kb + 1) * 128],
                        QTt[pb:pb + 64, qt * QT:(qt + 1) * QT],
                        start=True, stop=True)
                return sc

            pending_wo = []

            def emit_wo(qt, qc):
                po = ppv.tile([128, D], F32, tag="pvb", name=f"po{qt}_{qc}")
                nc.tensor.matmul(
                    po, xts[qt][:, qc * 128:(qc + 1) * 128], WO128,
                    start=True, stop=True)
                outt = late.tile([128, D], F32, tag="outt", bufs=3)
                nc.vector.tensor_copy(outt, po)
                nc.sync.dma_start(
                    out=out[qt * QT + qc * 128:qt * QT + (qc + 1) * 128, :],
                    in_=outt)

            flat = []
            for ui in range(len(units)):
                qt, h = units[ui]
                for gi, (kb0, gn) in enumerate(groups):
                    flat.append((ui, qt, h, gi, kb0, gn))

            sc_tiles = {}

            def emit_sc(idx):
                _, qt, h, _, kb0, gn = flat[idx]
                sc_tiles[idx] = emit_scores(qt, h, kb0, gn)

            emit_sc(0)
            emit_sc(1)
            pv = None
            pending_norm = None
            for idx, (ui, qt, h, gi, kb0, gn) in enumerate(flat):
                if h == 0 and gi == 0 and qt not in xts:
                    xts[qt] = pxt.tile([128, QT], BF16, tag="xt",
                                       name=f"xt{qt}")
                if h == 1 and gi in (0, 3, 6, 9) and qt + 2 < NQT:
                    load_mask(qt + 2, gi // 3, gi // 3 + 1)
                if gi == 0:
                    pv = ppv.tile([128, QT], F32, tag="pvb", name=f"pv{ui}")
                # drains BEFORE the score prefetch: drained projections must
                # precede, in emission order, any consumer of their outputs
                drain_task(ui, gi)
                if idx + 2 < len(flat):
                    emit_sc(idx + 2)
                sc = sc_tiles.pop(idx)
                ex = pex.tile([128, G, QT], BF16, tag="ex")
                nc.scalar.activation(ex[:, 0:gn, :], sc[:, 0:gn, :],
                                     EXP, scale=0.125)
                mk = pex.tile([128, G, QT], BF16, tag="mk")
                nc.vector.tensor_tensor(
                    mk[:, 0:gn, :], ex[:, 0:gn, :],
                    maskq[qt][:, kb0:kb0 + gn, :], op=MULT)
                for i in range(gn):
                    kb = kb0 + i
                    nc.tensor.matmul(
                        pv[0:65, :],
                        VA[:, kb, h * 65:(h + 1) * 65],
                        mk[:, i, :],
                        start=(kb == 0), stop=(kb == KB - 1))
                if gi == 4 and pending_norm is not None:
                    # deferred normalize-multiply of the previous unit (its
                    # pv slot frees here, mid-unit, so the boundary never
                    # serializes on the norm chain)
                    emit_norm2(*pending_norm)
                    pending_norm = None
                elif gi in (5, 6, 7, 8) and pending_wo:
                    emit_wo(*pending_wo.pop(0))
                if gi == len(groups) - 1:
                    if ui == len(units) - 1:
                        # final unit: fast-path norm via PE broadcast
                        # (score PSUM slots are free at this point)
                        rc = pwrk.tile([128, QT], F32, tag="rc", name="rcF")
                        nc.vector.reciprocal(rc[64:65, :], pv[64:65, :])
                        bct = psc.tile([128, G, QT], F32, tag="sc",
                                       name="bcF")
                        nc.tensor.matmul(bct[0:64, 0, :], ones_t[64:65, :],
                                         rc[64:65, :], start=True, stop=True)
                        bcs = pwrk.tile([64, QT], F32, tag="bcs", name="bcsF")
                        nc.vector.tensor_copy(bcs, bct[0:64, 0, :])
                        emit_norm2(qt, h, pv, bcs)
                        pending_wo.extend(
                            (qt, qc) for qc in range(QT // 128))
                        while pending_wo:
                            emit_wo(*pending_wo.pop(0))
                    else:
                        bcs = emit_norm1(ui, pv)
                        pending_norm = (qt, h, pv, bcs)
                        if h == HP - 1:
                            pending_wo.extend(
                                (qt, qc) for qc in range(QT // 128))

    nc.compile()
    nc.m = get_hw_module(nc.m)
    return nc


def _get_built():
    global _BUILT
    if _BUILT is None:
        _BUILT = _build()
    return _BUILT


def kernel(q, k, v, mask, w_q, w_k, w_v, w_o):
    import os
    # NTFF tracing needs antenv.axon_hooks, absent in some environments;
    # never let an inherited BASS_TRACE env var route us into that path.
    os.environ.setdefault("BASS_NEVER_TRACE", "1")
    import ml_dtypes
    from concourse.bass_utils import run_bass_kernel_spmd

    bf16 = ml_dtypes.bfloat16

    q = np.asarray(q, dtype=np.float32)
    k = np.asarray(k, dtype=np.float32)
    v = np.asarray(v, dtype=np.float32)
    mask = np.asarray(mask, dtype=np.int32)
    w_q = np.asarray(w_q, dtype=np.float32)
    w_k = np.asarray(w_k, dtype=np.float32)
    w_v = np.asarray(w_v, dtype=np.float32)
    w_o = np.asarray(w_o, dtype=np.float32)

    nc = _get_built()

    qT = [np.ascontiguousarray(q[b].T).astype(bf16) for b in range(B)]
    kT = [np.ascontiguousarray(k[b].T).astype(bf16) for b in range(B)]
    vT = [np.ascontiguousarray(v[b].T).astype(bf16) for b in range(B)]
    # maskP[qt, p, kb, q] = mask[b, qt*512+q, kb*128+p], pre-arranged so
    # each query tile's mask is one contiguous-run DMA
    maskP = [np.ascontiguousarray(
        mask[b].astype(np.uint8).reshape(NQT, QT, KB, 128)
        .transpose(0, 3, 2, 1)) for b in range(B)]

    in_maps = []
    for c in range(NCORES):
        b, hp = divmod(c, 4)
        cs = hp * HP * DK
        ce = cs + HP * DK
        in_maps.append({
            "qT": qT[b], "kT": kT[b], "vT": vT[b], "maskP": maskP[b],
            "wq": np.ascontiguousarray(w_q[:, cs:ce]).astype(bf16),
            "wk": np.ascontiguousarray(w_k[:, cs:ce]).astype(bf16),
            "wv": np.ascontiguousarray(w_v[:, cs:ce]).astype(bf16),
            "wo": np.ascontiguousarray(w_o[cs:ce, :]).astype(bf16),
        })

    global _LAST_IN_MAPS
    _LAST_IN_MAPS = in_maps
    res = run_bass_kernel_spmd(nc, in_maps, list(range(NCORES)))

    # Megatron row-parallel unshard: sum the 4 partial w_o contributions
    full = np.empty((B, S, D), dtype=np.float32)
    for b in range(B):
        acc = np.zeros((S, D), dtype=np.float32)
        for hp in range(4):
            acc += np.asarray(res.results[b * 4 + hp]["out"],
                              dtype=np.float32)
        full[b] = acc
    return full


# revision 17
# speedup vs baseline: 1.0980x; 1.0032x over previous
"""Multi-head attention kernel for Trainium2, 8 NeuronCores.

Problem: B=2, S=4096, D=512, H=8 heads (dk=64), explicit S x S masked softmax.

Sharding (Megatron-style tensor parallel): batch (2) x head-pairs (4) -> 8
cores. Each core computes 2 heads for ALL 4096 queries of one batch element,
with w_q/w_k/w_v column-sliced (128 cols) and w_o row-sliced (128 rows).
Each core emits a PARTIAL [S, D] output (its heads' w_o contribution); the
host sums the 4 partials per batch during unsharding. This removes the 4x
K/V projection duplication of a query-sharded layout: per-core projection
work drops from 180K to 49K PE cycles, so the serial projection prologue
shrinks ~4x and the steady state is ACT(exp)-limited.

Per-core layout choices:
  - scores computed transposed ([keys, q]) so the PV matmul consumes them
    directly (no on-chip transposes anywhere).
  - mask streamed per query-tile as uint8 and DMA-cast to bf16 on load.
  - softmax sums come from a ones-column appended to V (M=65 PV matmul);
    the reciprocal row is broadcast across partitions via a DRAM bounce
    (stride-0 partition DMA); normalize-multiplies are deferred into the
    next unit's pipeline so unit boundaries never stall.
  - both heads' normalized outputs are packed into one [128, QT] tile
    (DVE writes head 1 at partitions 64-127), so the output projection is
    a single 128-contraction matmul per 128 query rows.
  - ALL psum flows through two pools (scores ring 2x3 banks + pv/wo ring
    2x1): projections borrow score-ring slices, so leftover projection
    work (V quarters, Q tiles) drains into early attention groups where
    the PE has slack against the ACT exp rate.
  - all matmul operands bf16 (host- or DMA-cast), fp32 accumulation in PSUM.
"""

import numpy as np

B, S, D, H = 2, 4096, 512, 8
DK = D // H            # 64
NCORES = 8
HP = 2                 # heads per core
NQT = 8                # query tiles per core
QT = 512               # query tile
KBS = 128              # key block size
KB = S // KBS          # 32 key blocks
G = 3                  # key blocks per ACT exp group (3 PSUM banks, FD=1536)
QTR = 1024             # input staging quarter (columns)

_BUILT = None


def _build():
    import concourse.bacc as bacc
    import concourse.mybir as mybir
    import concourse.tile as tile
    from concourse.bass_interp import get_hw_module

    F32 = mybir.dt.float32
    BF16 = mybir.dt.bfloat16
    U8 = mybir.dt.uint8
    EXP = mybir.ActivationFunctionType.Exp
    MULT = mybir.AluOpType.mult

    nc = bacc.Bacc("TRN2", target_bir_lowering=False, debug=False,
                   enable_asserts=False, num_devices=NCORES)

    qT = nc.dram_tensor("qT", [D, S], BF16, kind="ExternalInput")
    kT = nc.dram_tensor("kT", [D, S], BF16, kind="ExternalInput")
    vT = nc.dram_tensor("vT", [D, S], BF16, kind="ExternalInput")
    maskP = nc.dram_tensor("maskP", [NQT, 128, KB, QT], U8,
                           kind="ExternalInput")
    wq = nc.dram_tensor("wq", [D, HP * DK], BF16, kind="ExternalInput")
    wk = nc.dram_tensor("wk", [D, HP * DK], BF16, kind="ExternalInput")
    wv = nc.dram_tensor("wv", [D, HP * DK], BF16, kind="ExternalInput")
    wo = nc.dram_tensor("wo", [HP * DK, D], BF16, kind="ExternalInput")
    out = nc.dram_tensor("out", [S, D], F32, kind="ExternalOutput")
    # DRAM bounce buffer for broadcasting softmax reciprocals across partitions
    rcd = nc.dram_tensor("rcd", [NQT * HP, 512], F32, kind="Internal")

    import concourse.bass as bass

    with tile.TileContext(nc) as tc:
        with tc.tile_pool(name="persist", bufs=1) as persist, \
             tc.tile_pool(name="maskp", bufs=2) as maskp, \
             tc.tile_pool(name="masku", bufs=2) as masku, \
             tc.tile_pool(name="pstg", bufs=3) as pstg, \
             tc.tile_pool(name="pxt", bufs=2) as pxt, \
             tc.tile_pool(name="pwrk", bufs=3) as pwrk, \
             tc.tile_pool(name="pex", bufs=4) as pex, \
             tc.tile_pool(name="late", bufs=1) as late, \
             tc.tile_pool(name="psc", bufs=2, space="PSUM") as psc, \
             tc.tile_pool(name="ppv", bufs=2, space="PSUM") as ppv:

            ppv._bctag = "pvb"

            # ---------------- persistent tiles ----------------
            KT = persist.tile([128, S], BF16)        # K^T (2 heads' dk rows)
            QTt = persist.tile([128, S], BF16)       # Q^T
            VA = persist.tile([128, KB, HP * 65], BF16)  # V + ones col/head
            ones_t = persist.tile([128, 64], F32)
            nc.vector.memset(ones_t, 1.0)
            WO128 = persist.tile([128, D], BF16)     # both heads' wo rows
            wk_b = persist.tile([128, 4, 128], BF16)
            wq_b = persist.tile([128, 4, 128], BF16)
            wv_b = persist.tile([128, 4, 128], BF16)

            va_ones = VA.rearrange("p kb (h x) -> p kb h x", x=65)[:, :, :, 64:65]
            nc.gpsimd.memset(va_ones, 1.0)

            kT_src = kT[:, :].rearrange("(c p) s -> p c s", p=128)
            vT_src = vT[:, :].rearrange("(c p) s -> p c s", p=128)
            qT_src = qT[:, :].rearrange("(c p) s -> p c s", p=128)

            maskq = {}
            masku_t = {}

            def get_mq(qt):
                if qt not in maskq:
                    maskq[qt] = maskp.tile([128, KB, QT], BF16, tag="mq",
                                           name=f"mq{qt}")
                return maskq[qt]

            def load_masku(qt, quarter):
                # raw uint8 quarter (half the DMA bytes of the casting path)
                if qt not in masku_t:
                    masku_t[qt] = masku.tile([128, KB, QT], U8, tag="mu",
                                             name=f"mu{qt}")
                nc.gpsimd.dma_start(
                    out=masku_t[qt][:, quarter * 8:(quarter + 1) * 8, :],
                    in_=maskP[qt, :, quarter * 8:(quarter + 1) * 8, :])

            def conv_mask(qt, quarter):
                # u8 -> bf16 on the otherwise-idle GPSIMD engine
                nc.gpsimd.tensor_copy(
                    get_mq(qt)[:, quarter * 8:(quarter + 1) * 8, :],
                    masku_t[qt][:, quarter * 8:(quarter + 1) * 8, :])

            def load_mask(qt, mc0=0, mc1=4):
                # 4 preemptible sub-DMAs so latency-sensitive transfers
                # (rcd bounce) interleave between them
                mq = get_mq(qt)
                for mc in range(4 * mc0, 4 * mc1):
                    nc.gpsimd.dma_start(
                        out=mq[:, mc * 2:(mc + 1) * 2, :],
                        in_=maskP[qt, :, mc * 2:(mc + 1) * 2, :])

            # ---------------- projections (quarter-staged) ----------------
            # All proj psum borrows score-ring tiles ([128, 3, 512] = 3
            # banks), so projections and attention share one PSUM layout.
            def stage(src, qtr, name):
                t = pstg.tile([128, 4, QTR], BF16, tag="stg", name=name)
                nc.gpsimd.dma_start(
                    out=t, in_=src[:, :, qtr * QTR:(qtr + 1) * QTR])
                return t

            def proj_kq(w_b, tin, dst, toff, nst, name):
                # dst[:, toff : toff+nst*512] from one staged quarter
                pt = psc.tile([128, G, QT], F32, tag="sc", name=f"pp{name}")
                for ns in range(nst):
                    for di in range(4):
                        nc.tensor.matmul(
                            pt[:, ns, :], w_b[:, di, :],
                            tin[:, di, ns * 512:(ns + 1) * 512],
                            start=(di == 0), stop=(di == 3))
                nc.vector.tensor_copy(
                    dst[:, toff:toff + nst * 512],
                    pt[:, 0:nst, :].rearrange("p a b -> p (a b)"))

            def proj_v(tin, sc0):
                # 8 VA chunks [128 s-rows, 128 v-cols] from one staged quarter
                pt = psc.tile([128, G, QT], F32, tag="sc", name=f"ppv{sc0}")
                for i in range(8):
                    si = (sc0 + i) % 8
                    for di in range(4):
                        nc.tensor.matmul(
                            pt[:, i // 4, (i % 4) * 128:(i % 4 + 1) * 128],
                            tin[:, di, si * 128:(si + 1) * 128],
                            wv_b[:, di, :],
                            start=(di == 0), stop=(di == 3))
                src = pt.rearrange("p g (qc hx) -> p (g qc) hx", hx=128)
                src = src.rearrange("p c (h x) -> p c h x", x=64)[:, 0:8]
                dst = VA[:, sc0:sc0 + 8, :].rearrange(
                    "p kb (h x) -> p kb h x", x=65)[:, :, :, 0:64]
                nc.vector.tensor_copy(dst, src)

            # prologue DMA order is latency-critical: first-needed first.
            # Only K quarter 0 / Q quarter 0 / mask quarter 0 / V quarter 0
            # are projected before attention emission starts; everything
            # else drains into early attention groups in consumption order.
            kq0 = stage(kT_src, 0, "kq0")
            nc.gpsimd.dma_start(
                out=wk_b, in_=wk[:, :].rearrange("(c p) d -> p c d", p=128))
            qq0 = stage(qT_src, 0, "qq0")
            nc.gpsimd.dma_start(
                out=wq_b, in_=wq[:, :].rearrange("(c p) d -> p c d", p=128))
            load_mask(0, 0, 1)
            vq0 = stage(vT_src, 0, "vq0")
            nc.gpsimd.dma_start(
                out=wv_b, in_=wv[:, :].rearrange("(c p) d -> p c d", p=128))
            proj_kq(wk_b, kq0, KT, 0, 2, "k0")
            kq1 = stage(kT_src, 1, "kq1")
            proj_kq(wq_b, qq0, QTt, 0, 2, "q0")
            load_mask(0, 1, 2)
            vq1 = stage(vT_src, 1, "vq1")
            staged = {("kq", 1): kq1, ("v", 0): vq0, ("v", 1): vq1}

            # drained actions, keyed by (unit, group) emission slots; each
            # runs on the shared psc ring / DMA queues in consumption order
            def a_dma_kq(qtr):
                return lambda: staged.__setitem__(
                    ("kq", qtr), stage(kT_src, qtr, f"kq{qtr}"))

            def a_dma_vq(qtr):
                return lambda: staged.__setitem__(
                    ("v", qtr), stage(vT_src, qtr, f"vq{qtr}"))

            def a_dma_qq(qtr):
                return lambda: staged.__setitem__(
                    ("q", qtr), stage(qT_src, qtr, f"qq{qtr}"))

            def a_k(qtr):
                return lambda: proj_kq(wk_b, staged.pop(("kq", qtr)), KT,
                                       qtr * QTR, 2, f"k{qtr}")

            def a_v(qtr):
                return lambda: proj_v(staged.pop(("v", qtr)), qtr * 8)

            def a_q(qtr):
                return lambda: proj_kq(wq_b, staged.pop(("q", qtr)), QTt,
                                       qtr * QTR, 2, f"q{qtr}")

            def a_m0(quarter):
                return lambda: load_mask(0, quarter, quarter + 1)

            def a_m1(quarter):
                return lambda: load_mask(1, quarter, quarter + 1)

            def a_wo():
                return lambda: nc.gpsimd.dma_start(out=WO128, in_=wo[:, :])

            drain_slots = {
                (0, 0): [a_k(1), a_v(0), a_m0(2)],
                (0, 1): [a_dma_kq(2), a_dma_vq(2)],
                (0, 2): [a_v(1), a_m0(3)],
                (0, 3): [a_k(2)],
                (0, 5): [a_v(2), a_dma_kq(3), a_dma_vq(3)],
                (0, 6): [a_k(3)],
                (0, 8): [a_v(3)],
                (0, 10): [a_wo()],
                (1, 1): [a_dma_qq(1)],
                (2, 5): [a_q(1)],
                (2, 7): [a_dma_qq(2)],
                (3, 5): [a_q(2)],
                (3, 7): [a_dma_qq(3)],
                (4, 5): [a_q(3)],
            }

            def drain_task(ui, gi):
                for a in drain_slots.get((ui, gi), ()):
                    a()

            # ---------------- attention ----------------
            units = [(qt, h) for qt in range(NQT) for h in range(HP)]
            xts = {}

            def emit_norm1(ui, pv):
                # reciprocal of the sums row, bounced through DRAM to
                # broadcast it across partitions 0-63 (SBUF APs cannot
                # have a zero partition stride; DRAM APs can). The SP
                # queue carries only this + the small out-writes, so the
                # latency-sensitive bounce never queues behind bulk DMAs.
                rc = pwrk.tile([128, QT], F32, tag="rc", name=f"rc{ui}")
                nc.vector.reciprocal(rc[64:65, :], pv[64:65, :])
                nc.sync.dma_start(out=rcd[ui:ui + 1, :], in_=rc[64:65, :])
                bcs = pwrk.tile([64, QT], F32, tag="bcs", name=f"bcs{ui}")
                src = rcd[ui:ui + 1, :]
                bsrc = bass.AP(tensor=src.tensor, offset=src.offset,
                               ap=[[0, 64]] + [list(a) for a in src.ap[1:]])
                nc.sync.dma_start(out=bcs, in_=bsrc)
                return bcs

            def emit_norm2(qt, h, pv, bcs):
                # head h lands at partitions h*64 .. h*64+63 (packed xt)
                nc.vector.tensor_tensor(
                    xts[qt][h * 64:(h + 1) * 64, :], pv[0:64, :], bcs, op=MULT)

            groups = []
            kb0 = 0
            while kb0 < KB:
                groups.append((kb0, min(G, KB - kb0)))
                kb0 += G

            def emit_scores(qt, h, kb0, gn):
                pb = h * 64
                sc = psc.tile([128, G, QT], F32, tag="sc")
                for i in range(gn):
                    kb = kb0 + i
                    nc.tensor.matmul(
                        sc[:, i, :],
                        KT[pb:pb + 64, kb * 128:(kb + 1) * 128],
                        QTt[pb:pb + 64, qt * QT:(qt + 1) * QT],
                        start=True, stop=True)
                return sc

            pending_wo = []

            def emit_wo(qt, qc):
                po = ppv.tile([128, D], F32, tag="pvb", name=f"po{qt}_{qc}")
                nc.tensor.matmul(
                    po, xts[qt][:, qc * 128:(qc + 1) * 128], WO128,
                    start=True, stop=True)
                outt = late.tile([128, D], F32, tag="outt", bufs=3)
                nc.vector.tensor_copy(outt, po)
                nc.sync.dma_start(
                    out=out[qt * QT + qc * 128:qt * QT + (qc + 1) * 128, :],
                    in_=outt)

            flat = []
            for ui in range(len(units)):
                qt, h = units[ui]
                for gi, (kb0, gn) in enumerate(groups):
                    flat.append((ui, qt, h, gi, kb0, gn))

            sc_tiles = {}

            def emit_sc(idx):
                _, qt, h, _, kb0, gn = flat[idx]
                sc_tiles[idx] = emit_scores(qt, h, kb0, gn)

            emit_sc(0)
            emit_sc(1)
            pv = None
            pending_norm = None
            for idx, (ui, qt, h, gi, kb0, gn) in enumerate(flat):
                if h == 0 and gi == 0 and qt not in xts:
                    xts[qt] = pxt.tile([128, QT], BF16, tag="xt",
                                       name=f"xt{qt}")
                if h == 0 and gi in (0, 3, 6, 9) and qt + 1 < NQT:
                    load_masku(qt + 1, gi // 3)
                if h == 1 and gi in (1, 4, 7, 10) and qt + 1 < NQT:
                    conv_mask(qt + 1, {1: 0, 4: 1, 7: 2, 10: 3}[gi])
                if gi == 0:
                    pv = ppv.tile([128, QT], F32, tag="pvb", name=f"pv{ui}")
                # drains BEFORE the score prefetch: drained projections must
                # precede, in emission order, any consumer of their outputs
                drain_task(ui, gi)
                if idx + 2 < len(flat):
                    emit_sc(idx + 2)
                sc = sc_tiles.pop(idx)
                ex = pex.tile([128, G, QT], BF16, tag="ex")
                nc.scalar.activation(ex[:, 0:gn, :], sc[:, 0:gn, :],
                                     EXP, scale=0.125)
                mk = pex.tile([128, G, QT], BF16, tag="mk")
                nc.vector.tensor_tensor(
                    mk[:, 0:gn, :], ex[:, 0:gn, :],
                    maskq[qt][:, kb0:kb0 + gn, :], op=MULT)
                for i in range(gn):
                    kb = kb0 + i
                    nc.tensor.matmul(
                        pv[0:65, :],
                        VA[:, kb, h * 65:(h + 1) * 65],
                        mk[:, i, :],
                        start=(kb == 0), stop=(kb == KB - 1))
                if gi == 4 and pending_norm is not None:
                    # deferred normalize-multiply of the previous unit (its
                    # pv slot frees here, mid-unit, so the boundary never
                    # serializes on the norm chain)
                    emit_norm2(*pending_norm)
                    pending_norm = None
                elif gi in (5, 6, 7, 8) and pending_wo:
                    emit_wo(*pending_wo.pop(0))
                if gi == len(groups) - 1:
                    if ui == len(units) - 1:
                        # final unit: fast-path norm via PE broadcast
                        # (score PSUM slots are free at this point)
                        rc = pwrk.tile([128, QT], F32, tag="rc", name="rcF")
                        nc.vector.reciprocal(rc[64:65, :], pv[64:65, :])
                        bct = psc.tile([128, G, QT], F32, tag="sc",
                                       name="bcF")
                        nc.tensor.matmul(bct[0:64, 0, :], ones_t[64:65, :],
                                         rc[64:65, :], start=True, stop=True)
                        bcs = pwrk.tile([64, QT], F32, tag="bcs", name="bcsF")
                        nc.vector.tensor_copy(bcs, bct[0:64, 0, :])
                        emit_norm2(qt, h, pv, bcs)
                        pending_wo.extend(
                            (qt, qc) for qc in range(QT // 128))
                        while pending_wo:
                            emit_wo(*pending_wo.pop(0))
                    else:
                        bcs = emit_norm1(ui, pv)
                        pending_norm = (qt, h, pv, bcs)
                        if h == HP - 1:
                            pending_wo.extend(
                                (qt, qc) for qc in range(QT // 128))

    nc.compile()
    nc.m = get_hw_module(nc.m)
    return nc


def _get_built():
    global _BUILT
    if _BUILT is None:
        _BUILT = _build()
    return _BUILT


def kernel(q, k, v, mask, w_q, w_k, w_v, w_o):
    import os
    # NTFF tracing needs antenv.axon_hooks, absent in some environments;
    # never let an inherited BASS_TRACE env var route us into that path.
    os.environ.setdefault("BASS_NEVER_TRACE", "1")
    import ml_dtypes
    from concourse.bass_utils import run_bass_kernel_spmd

    bf16 = ml_dtypes.bfloat16

    q = np.asarray(q, dtype=np.float32)
    k = np.asarray(k, dtype=np.float32)
    v = np.asarray(v, dtype=np.float32)
    mask = np.asarray(mask, dtype=np.int32)
    w_q = np.asarray(w_q, dtype=np.float32)
    w_k = np.asarray(w_k, dtype=np.float32)
    w_v = np.asarray(w_v, dtype=np.float32)
    w_o = np.asarray(w_o, dtype=np.float32)

    nc = _get_built()

    qT = [np.ascontiguousarray(q[b].T).astype(bf16) for b in range(B)]
    kT = [np.ascontiguousarray(k[b].T).astype(bf16) for b in range(B)]
    vT = [np.ascontiguousarray(v[b].T).astype(bf16) for b in range(B)]
    # maskP[qt, p, kb, q] = mask[b, qt*512+q, kb*128+p], pre-arranged so
    # each query tile's mask is one contiguous-run DMA
    maskP = [np.ascontiguousarray(
        mask[b].astype(np.uint8).reshape(NQT, QT, KB, 128)
        .transpose(0, 3, 2, 1)) for b in range(B)]

    in_maps = []
    for c in range(NCORES):
        b, hp = divmod(c, 4)
        cs = hp * HP * DK
        ce = cs + HP * DK
        in_maps.append({
            "qT": qT[b], "kT": kT[b], "vT": vT[b], "maskP": maskP[b],
            "wq": np.ascontiguousarray(w_q[:, cs:ce]).astype(bf16),
            "wk": np.ascontiguousarray(w_k[:, cs:ce]).astype(bf16),
            "wv": np.ascontiguousarray(w_v[:, cs:ce]).astype(bf16),
            "wo": np.ascontiguousarray(w_o[cs:ce, :]).astype(bf16),
        })

    global _LAST_IN_MAPS
    _LAST_IN_MAPS = in_maps
    res = run_bass_kernel_spmd(nc, in_maps, list(range(NCORES)))

    # Megatron row-parallel unshard: sum the 4 partial w_o contributions
    full = np.empty((B, S, D), dtype=np.float32)
    for b in range(B):
        acc = np.zeros((S, D), dtype=np.float32)
        for hp in range(4):
            acc += np.asarray(res.results[b * 4 + hp]["out"],
                              dtype=np.float32)
        full[b] = acc
    return full


# revision 22
# speedup vs baseline: 1.0983x; 1.0002x over previous
"""Multi-head attention kernel for Trainium2, 8 NeuronCores.

Problem: B=2, S=4096, D=512, H=8 heads (dk=64), explicit S x S masked softmax.

Sharding (Megatron-style tensor parallel): batch (2) x head-pairs (4) -> 8
cores. Each core computes 2 heads for ALL 4096 queries of one batch element,
with w_q/w_k/w_v column-sliced (128 cols) and w_o row-sliced (128 rows).
Each core emits a PARTIAL [S, D] output (its heads' w_o contribution); the
host sums the 4 partials per batch during unsharding. This removes the 4x
K/V projection duplication of a query-sharded layout: per-core projection
work drops from 180K to 49K PE cycles, so the serial projection prologue
shrinks ~4x and the steady state is ACT(exp)-limited.

Per-core layout choices:
  - scores computed transposed ([keys, q]) so the PV matmul consumes them
    directly (no on-chip transposes anywhere).
  - mask streamed per query-tile as uint8 and DMA-cast to bf16 on load.
  - softmax sums come from a ones-column appended to V (M=65 PV matmul);
    the reciprocal row is broadcast across partitions via a DRAM bounce
    (stride-0 partition DMA); normalize-multiplies are deferred into the
    next unit's pipeline so unit boundaries never stall.
  - both heads' normalized outputs are packed into one [128, QT] tile
    (DVE writes head 1 at partitions 64-127), so the output projection is
    a single 128-contraction matmul per 128 query rows.
  - ALL psum flows through two pools (scores ring 2x3 banks + pv/wo ring
    2x1): projections borrow score-ring slices, so leftover projection
    work (V quarters, Q tiles) drains into early attention groups where
    the PE has slack against the ACT exp rate.
  - all matmul operands bf16 (host- or DMA-cast), fp32 accumulation in PSUM.
"""

import numpy as np

B, S, D, H = 2, 4096, 512, 8
DK = D // H            # 64
NCORES = 8
HP = 2                 # heads per core
NQT = 8                # query tiles per core
QT = 512               # query tile
KBS = 128              # key block size
KB = S // KBS          # 32 key blocks
G = 3                  # key blocks per ACT exp group (3 PSUM banks, FD=1536)
QTR = 1024             # input staging quarter (columns)

_BUILT = None


def _build():
    import concourse.bacc as bacc
    import concourse.mybir as mybir
    import concourse.tile as tile
    from concourse.bass_interp import get_hw_module

    F32 = mybir.dt.float32
    BF16 = mybir.dt.bfloat16
    U8 = mybir.dt.uint8
    EXP = mybir.ActivationFunctionType.Exp
    MULT = mybir.AluOpType.mult

    nc = bacc.Bacc("TRN2", target_bir_lowering=False, debug=False,
                   enable_asserts=False, num_devices=NCORES)

    qT = nc.dram_tensor("qT", [D, S], BF16, kind="ExternalInput")
    kT = nc.dram_tensor("kT", [D, S], BF16, kind="ExternalInput")
    vT = nc.dram_tensor("vT", [D, S], BF16, kind="ExternalInput")
    maskP = nc.dram_tensor("maskP", [NQT, 128, KB, QT], U8,
                           kind="ExternalInput")
    wq = nc.dram_tensor("wq", [D, HP * DK], BF16, kind="ExternalInput")
    wk = nc.dram_tensor("wk", [D, HP * DK], BF16, kind="ExternalInput")
    wv = nc.dram_tensor("wv", [D, HP * DK], BF16, kind="ExternalInput")
    wo = nc.dram_tensor("wo", [HP * DK, D], BF16, kind="ExternalInput")
    out = nc.dram_tensor("out", [S, D], F32, kind="ExternalOutput")
    # DRAM bounce buffer for broadcasting softmax reciprocals across partitions
    rcd = nc.dram_tensor("rcd", [NQT * HP, 512], F32, kind="Internal")

    import concourse.bass as bass

    with tile.TileContext(nc) as tc:
        with tc.tile_pool(name="persist", bufs=1) as persist, \
             tc.tile_pool(name="maskp", bufs=2) as maskp, \
             tc.tile_pool(name="masku", bufs=2) as masku, \
             tc.tile_pool(name="pstg", bufs=3) as pstg, \
             tc.tile_pool(name="pxt", bufs=2) as pxt, \
             tc.tile_pool(name="pwrk", bufs=3) as pwrk, \
             tc.tile_pool(name="pex", bufs=6) as pex, \
             tc.tile_pool(name="late", bufs=1) as late, \
             tc.tile_pool(name="psc", bufs=2, space="PSUM") as psc, \
             tc.tile_pool(name="ppv", bufs=2, space="PSUM") as ppv:

            ppv._bctag = "pvb"

            # ---------------- persistent tiles ----------------
            KT = persist.tile([128, S], BF16)        # K^T (2 heads' dk rows)
            QTt = persist.tile([128, S], BF16)       # Q^T
            VA = persist.tile([128, KB, HP * 65], BF16)  # V + ones col/head
            ones_t = persist.tile([128, 64], F32)
            nc.vector.memset(ones_t, 1.0)
            WO128 = persist.tile([128, D], BF16)     # both heads' wo rows
            wk_b = persist.tile([128, 4, 128], BF16)
            wq_b = persist.tile([128, 4, 128], BF16)
            wv_b = persist.tile([128, 4, 128], BF16)

            va_ones = VA.rearrange("p kb (h x) -> p kb h x", x=65)[:, :, :, 64:65]
            nc.gpsimd.memset(va_ones, 1.0)

            kT_src = kT[:, :].rearrange("(c p) s -> p c s", p=128)
            vT_src = vT[:, :].rearrange("(c p) s -> p c s", p=128)
            qT_src = qT[:, :].rearrange("(c p) s -> p c s", p=128)

            maskq = {}
            masku_t = {}

            def get_mq(qt):
                if qt not in maskq:
                    maskq[qt] = maskp.tile([128, KB, QT], BF16, tag="mq",
                                           name=f"mq{qt}")
                return maskq[qt]

            def load_masku(qt, quarter):
                # raw uint8 quarter (half the DMA bytes of the casting path)
                if qt not in masku_t:
                    masku_t[qt] = masku.tile([128, KB, QT], U8, tag="mu",
                                             name=f"mu{qt}")
                nc.gpsimd.dma_start(
                    out=masku_t[qt][:, quarter * 8:(quarter + 1) * 8, :],
                    in_=maskP[qt, :, quarter * 8:(quarter + 1) * 8, :])

            def conv_mask(qt, quarter):
                # u8 -> bf16 on the otherwise-idle GPSIMD engine
                nc.gpsimd.tensor_copy(
                    get_mq(qt)[:, quarter * 8:(quarter + 1) * 8, :],
                    masku_t[qt][:, quarter * 8:(quarter + 1) * 8, :])

            def load_mask(qt, mc0=0, mc1=4):
                # 4 preemptible sub-DMAs so latency-sensitive transfers
                # (rcd bounce) interleave between them
                mq = get_mq(qt)
                for mc in range(4 * mc0, 4 * mc1):
                    nc.gpsimd.dma_start(
                        out=mq[:, mc * 2:(mc + 1) * 2, :],
                        in_=maskP[qt, :, mc * 2:(mc + 1) * 2, :])

            # ---------------- projections (quarter-staged) ----------------
            # All proj psum borrows score-ring tiles ([128, 3, 512] = 3
            # banks), so projections and attention share one PSUM layout.
            def stage(src, qtr, name):
                t = pstg.tile([128, 4, QTR], BF16, tag="stg", name=name)
                nc.gpsimd.dma_start(
                    out=t, in_=src[:, :, qtr * QTR:(qtr + 1) * QTR])
                return t

            def proj_kq(w_b, tin, dst, toff, nst, name):
                # dst[:, toff : toff+nst*512] from one staged quarter
                pt = psc.tile([128, G, QT], F32, tag="sc", name=f"pp{name}")
                for ns in range(nst):
                    for di in range(4):
                        nc.tensor.matmul(
                            pt[:, ns, :], w_b[:, di, :],
                            tin[:, di, ns * 512:(ns + 1) * 512],
                            start=(di == 0), stop=(di == 3))
                nc.vector.tensor_copy(
                    dst[:, toff:toff + nst * 512],
                    pt[:, 0:nst, :].rearrange("p a b -> p (a b)"))

            def proj_v(tin, sc0):
                # 8 VA chunks [128 s-rows, 128 v-cols] from one staged quarter
                pt = psc.tile([128, G, QT], F32, tag="sc", name=f"ppv{sc0}")
                for i in range(8):
                    si = (sc0 + i) % 8
                    for di in range(4):
                        nc.tensor.matmul(
                            pt[:, i // 4, (i % 4) * 128:(i % 4 + 1) * 128],
                            tin[:, di, si * 128:(si + 1) * 128],
                            wv_b[:, di, :],
                            start=(di == 0), stop=(di == 3))
                src = pt.rearrange("p g (qc hx) -> p (g qc) hx", hx=128)
                src = src.rearrange("p c (h x) -> p c h x", x=64)[:, 0:8]
                dst = VA[:, sc0:sc0 + 8, :].rearrange(
                    "p kb (h x) -> p kb h x", x=65)[:, :, :, 0:64]
                nc.vector.tensor_copy(dst, src)

            # prologue DMA order is latency-critical: first-needed first.
            # Only K quarter 0 / Q quarter 0 / mask quarter 0 / V quarter 0
            # are projected before attention emission starts; everything
            # else drains into early attention groups in consumption order.
            kq0 = stage(kT_src, 0, "kq0")
            nc.gpsimd.dma_start(
                out=wk_b, in_=wk[:, :].rearrange("(c p) d -> p c d", p=128))
            qq0 = stage(qT_src, 0, "qq0")
            nc.gpsimd.dma_start(
                out=wq_b, in_=wq[:, :].rearrange("(c p) d -> p c d", p=128))
            load_mask(0, 0, 1)
            vq0 = stage(vT_src, 0, "vq0")
            nc.gpsimd.dma_start(
                out=wv_b, in_=wv[:, :].rearrange("(c p) d -> p c d", p=128))
            proj_kq(wk_b, kq0, KT, 0, 2, "k0")
            kq1 = stage(kT_src, 1, "kq1")
            proj_kq(wq_b, qq0, QTt, 0, 2, "q0")
            load_mask(0, 1, 2)
            vq1 = stage(vT_src, 1, "vq1")
            staged = {("kq", 1): kq1, ("v", 0): vq0, ("v", 1): vq1}

            # drained actions, keyed by (unit, group) emission slots; each
            # runs on the shared psc ring / DMA queues in consumption order
            def a_dma_kq(qtr):
                return lambda: staged.__setitem__(
                    ("kq", qtr), stage(kT_src, qtr, f"kq{qtr}"))

            def a_dma_vq(qtr):
                return lambda: staged.__setitem__(
                    ("v", qtr), stage(vT_src, qtr, f"vq{qtr}"))

            def a_dma_qq(qtr):
                return lambda: staged.__setitem__(
                    ("q", qtr), stage(qT_src, qtr, f"qq{qtr}"))

            def a_k(qtr):
                return lambda: proj_kq(wk_b, staged.pop(("kq", qtr)), KT,
                                       qtr * QTR, 2, f"k{qtr}")

            def a_v(qtr):
                return lambda: proj_v(staged.pop(("v", qtr)), qtr * 8)

            def a_q(qtr):
                return lambda: proj_kq(wq_b, staged.pop(("q", qtr)), QTt,
                                       qtr * QTR, 2, f"q{qtr}")

            def a_m0(quarter):
                return lambda: load_mask(0, quarter, quarter + 1)

            def a_m1(quarter):
                return lambda: load_mask(1, quarter, quarter + 1)

            def a_wo():
                return lambda: nc.gpsimd.dma_start(out=WO128, in_=wo[:, :])

            drain_slots = {
                (0, 0): [a_k(1), a_v(0), a_m0(2)],
                (0, 1): [a_dma_kq(2), a_dma_vq(2)],
                (0, 2): [a_v(1), a_m0(3)],
                (0, 3): [a_k(2)],
                (0, 5): [a_v(2), a_dma_kq(3), a_dma_vq(3)],
                (0, 6): [a_k(3)],
                (0, 8): [a_v(3)],
                (0, 10): [a_wo()],
                (1, 1): [a_dma_qq(1)],
                (2, 5): [a_q(1)],
                (2, 7): [a_dma_qq(2)],
                (3, 5): [a_q(2)],
                (3, 7): [a_dma_qq(3)],
                (4, 5): [a_q(3)],
            }

            def drain_task(ui, gi):
                for a in drain_slots.get((ui, gi), ()):
                    a()

            # ---------------- attention ----------------
            units = [(qt, h) for qt in range(NQT) for h in range(HP)]
            xts = {}

            def emit_norm1(ui, pv):
                # reciprocal of the sums row, bounced through DRAM to
                # broadcast it across partitions 0-63 (SBUF APs cannot
                # have a zero partition stride; DRAM APs can). The SP
                # queue carries only this + the small out-writes, so the
                # latency-sensitive bounce never queues behind bulk DMAs.
                rc = pwrk.tile([128, QT], F32, tag="rc", name=f"rc{ui}")
                nc.vector.reciprocal(rc[64:65, :], pv[64:65, :])
                nc.sync.dma_start(out=rcd[ui:ui + 1, :], in_=rc[64:65, :])
                bcs = pwrk.tile([64, QT], F32, tag="bcs", name=f"bcs{ui}")
                src = rcd[ui:ui + 1, :]
                bsrc = bass.AP(tensor=src.tensor, offset=src.offset,
                               ap=[[0, 64]] + [list(a) for a in src.ap[1:]])
                nc.sync.dma_start(out=bcs, in_=bsrc)
                return bcs

            def emit_norm2(qt, h, pv, bcs):
                # head h lands at partitions h*64 .. h*64+63 (packed xt)
                nc.vector.tensor_tensor(
                    xts[qt][h * 64:(h + 1) * 64, :], pv[0:64, :], bcs, op=MULT)

            groups = []
            kb0 = 0
            while kb0 < KB:
                groups.append((kb0, min(G, KB - kb0)))
                kb0 += G

            def emit_scores(qt, h, kb0, gn):
                pb = h * 64
                sc = psc.tile([128, G, QT], F32, tag="sc")
                for i in range(gn):
                    kb = kb0 + i
                    nc.tensor.matmul(
                        sc[:, i, :],
                        KT[pb:pb + 64, kb * 128:(kb + 1) * 128],
                        QTt[pb:pb + 64, qt * QT:(qt + 1) * QT],
                        start=True, stop=True)
                return sc

            pending_wo = []

            def emit_wo(qt, qc):
                po = ppv.tile([128, D], F32, tag="pvb", name=f"po{qt}_{qc}")
                nc.tensor.matmul(
                    po, xts[qt][:, qc * 128:(qc + 1) * 128], WO128,
                    start=True, stop=True)
                outt = late.tile([128, D], F32, tag="outt", bufs=3)
                nc.vector.tensor_copy(outt, po)
                nc.sync.dma_start(
                    out=out[qt * QT + qc * 128:qt * QT + (qc + 1) * 128, :],
                    in_=outt)

            flat = []
            for ui in range(len(units)):
                qt, h = units[ui]
                for gi, (kb0, gn) in enumerate(groups):
                    flat.append((ui, qt, h, gi, kb0, gn))

            sc_tiles = {}

            def emit_sc(idx):
                _, qt, h, _, kb0, gn = flat[idx]
                sc_tiles[idx] = emit_scores(qt, h, kb0, gn)

            emit_sc(0)
            emit_sc(1)
            pv = None
            pending_norm = None
            for idx, (ui, qt, h, gi, kb0, gn) in enumerate(flat):
                if h == 0 and gi == 0 and qt not in xts:
                    xts[qt] = pxt.tile([128, QT], BF16, tag="xt",
                                       name=f"xt{qt}")
                if h == 0 and gi in (0, 3, 6, 9) and qt + 1 < NQT:
                    load_masku(qt + 1, gi // 3)
                if h == 1 and gi in (1, 4, 7, 10) and qt + 1 < NQT:
                    conv_mask(qt + 1, {1: 0, 4: 1, 7: 2, 10: 3}[gi])
                if gi == 0:
                    pv = ppv.tile([128, QT], F32, tag="pvb", name=f"pv{ui}")
                # drains BEFORE the score prefetch: drained projections must
                # precede, in emission order, any consumer of their outputs
                drain_task(ui, gi)
                if idx + 2 < len(flat):
                    emit_sc(idx + 2)
                sc = sc_tiles.pop(idx)
                ex = pex.tile([128, G, QT], BF16, tag="ex")
                nc.scalar.activation(ex[:, 0:gn, :], sc[:, 0:gn, :],
                                     EXP, scale=0.125)
                mk = pex.tile([128, G, QT], BF16, tag="mk")
                nc.vector.tensor_tensor(
                    mk[:, 0:gn, :], ex[:, 0:gn, :],
                    maskq[qt][:, kb0:kb0 + gn, :], op=MULT)
                for i in range(gn):
                    kb = kb0 + i
                    nc.tensor.matmul(
                        pv[0:65, :],
                        VA[:, kb, h * 65:(h + 1) * 65],
                        mk[:, i, :],
                        start=(kb == 0), stop=(kb == KB - 1))
                if gi == 4 and pending_norm is not None:
                    # deferred normalize-multiply of the previous unit (its
                    # pv slot frees here, mid-unit, so the boundary never
                    # serializes on the norm chain)
                    emit_norm2(*pending_norm)
                    pending_norm = None
                elif gi in (5, 6, 7, 8) and pending_wo:
                    emit_wo(*pending_wo.pop(0))
                if gi == len(groups) - 1:
                    if ui == len(units) - 1:
                        # final unit: fast-path norm via PE broadcast
                        # (score PSUM slots are free at this point)
                        rc = pwrk.tile([128, QT], F32, tag="rc", name="rcF")
                        nc.vector.reciprocal(rc[64:65, :], pv[64:65, :])
                        bct = psc.tile([128, G, QT], F32, tag="sc",
                                       name="bcF")
                        nc.tensor.matmul(bct[0:64, 0, :], ones_t[64:65, :],
                                         rc[64:65, :], start=True, stop=True)
                        bcs = pwrk.tile([64, QT], F32, tag="bcs", name="bcsF")
                        nc.vector.tensor_copy(bcs, bct[0:64, 0, :])
                        emit_norm2(qt, h, pv, bcs)
                        pending_wo.extend(
                            (qt, qc) for qc in range(QT // 128))
                        while pending_wo:
                            emit_wo(*pending_wo.pop(0))
                    else:
                        bcs = emit_norm1(ui, pv)
                        pending_norm = (qt, h, pv, bcs)
                        if h == HP - 1:
                            pending_wo.extend(
                                (qt, qc) for qc in range(QT // 128))

    nc.compile()
    nc.m = get_hw_module(nc.m)
    return nc


def _get_built():
    global _BUILT
    if _BUILT is None:
        _BUILT = _build()
    return _BUILT


def kernel(q, k, v, mask, w_q, w_k, w_v, w_o):
    import os
    # NTFF tracing needs antenv.axon_hooks, absent in some environments;
    # never let an inherited BASS_TRACE env var route us into that path.
    os.environ.setdefault("BASS_NEVER_TRACE", "1")
    import ml_dtypes
    from concourse.bass_utils import run_bass_kernel_spmd

    bf16 = ml_dtypes.bfloat16

    q = np.asarray(q, dtype=np.float32)
    k = np.asarray(k, dtype=np.float32)
    v = np.asarray(v, dtype=np.float32)
    mask = np.asarray(mask, dtype=np.int32)
    w_q = np.asarray(w_q, dtype=np.float32)
    w_k = np.asarray(w_k, dtype=np.float32)
    w_v = np.asarray(w_v, dtype=np.float32)
    w_o = np.asarray(w_o, dtype=np.float32)

    nc = _get_built()

    qT = [np.ascontiguousarray(q[b].T).astype(bf16) for b in range(B)]
    kT = [np.ascontiguousarray(k[b].T).astype(bf16) for b in range(B)]
    vT = [np.ascontiguousarray(v[b].T).astype(bf16) for b in range(B)]
    # maskP[qt, p, kb, q] = mask[b, qt*512+q, kb*128+p], pre-arranged so
    # each query tile's mask is one contiguous-run DMA
    maskP = [np.ascontiguousarray(
        mask[b].astype(np.uint8).reshape(NQT, QT, KB, 128)
        .transpose(0, 3, 2, 1)) for b in range(B)]

    in_maps = []
    for c in range(NCORES):
        b, hp = divmod(c, 4)
        cs = hp * HP * DK
        ce = cs + HP * DK
        in_maps.append({
            "qT": qT[b], "kT": kT[b], "vT": vT[b], "maskP": maskP[b],
            "wq": np.ascontiguousarray(w_q[:, cs:ce]).astype(bf16),
            "wk": np.ascontiguousarray(w_k[:, cs:ce]).astype(bf16),
            "wv": np.ascontiguousarray(w_v[:, cs:ce]).astype(bf16),
            "wo": np.ascontiguousarray(w_o[cs:ce, :]).astype(bf16),
        })

    global _LAST_IN_MAPS
    _LAST_IN_MAPS = in_maps
    res = run_bass_kernel_spmd(nc, in_maps, list(range(NCORES)))

    # Megatron row-parallel unshard: sum the 4 partial w_o contributions
    full = np.empty((B, S, D), dtype=np.float32)
    for b in range(B):
        acc = np.zeros((S, D), dtype=np.float32)
        for hp in range(4):
            acc += np.asarray(res.results[b * 4 + hp]["out"],
                              dtype=np.float32)
        full[b] = acc
    return full


# revision 23
# speedup vs baseline: 1.1134x; 1.0138x over previous
"""Multi-head attention kernel for Trainium2, 8 NeuronCores.

Problem: B=2, S=4096, D=512, H=8 heads (dk=64), explicit S x S masked softmax.

Sharding (Megatron-style tensor parallel): batch (2) x head-pairs (4) -> 8
cores. Each core computes 2 heads for ALL 4096 queries of one batch element,
with w_q/w_k/w_v column-sliced (128 cols) and w_o row-sliced (128 rows).
Each core emits a PARTIAL [S, D] output (its heads' w_o contribution); the
host sums the 4 partials per batch during unsharding. This removes the 4x
K/V projection duplication of a query-sharded layout: per-core projection
work drops from 180K to 49K PE cycles, so the serial projection prologue
shrinks ~4x and the steady state is ACT(exp)-limited.

Per-core layout choices:
  - scores computed transposed ([keys, q]) so the PV matmul consumes them
    directly (no on-chip transposes anywhere).
  - mask streamed per query-tile as uint8 and DMA-cast to bf16 on load.
  - softmax sums come from a ones-column appended to V (M=65 PV matmul);
    the reciprocal row is broadcast across partitions via a DRAM bounce
    (stride-0 partition DMA); normalize-multiplies are deferred into the
    next unit's pipeline so unit boundaries never stall.
  - both heads' normalized outputs are packed into one [128, QT] tile
    (DVE writes head 1 at partitions 64-127), so the output projection is
    a single 128-contraction matmul per 128 query rows.
  - ALL psum flows through two pools (scores ring 2x3 banks + pv/wo ring
    2x1): projections borrow score-ring slices, so leftover projection
    work (V quarters, Q tiles) drains into early attention groups where
    the PE has slack against the ACT exp rate.
  - all matmul operands bf16 (host- or DMA-cast), fp32 accumulation in PSUM.
"""

import numpy as np

B, S, D, H = 2, 4096, 512, 8
DK = D // H            # 64
NCORES = 8
HP = 2                 # heads per core
NQT = 8                # query tiles per core
QT = 512               # query tile
KBS = 128              # key block size
KB = S // KBS          # 32 key blocks
G = 3                  # key blocks per ACT exp group (3 PSUM banks, FD=1536)
QTR = 1024             # input staging quarter (columns)

_BUILT = None


def _build():
    import concourse.bacc as bacc
    import concourse.mybir as mybir
    import concourse.tile as tile
    from concourse.bass_interp import get_hw_module

    F32 = mybir.dt.float32
    BF16 = mybir.dt.bfloat16
    U8 = mybir.dt.uint8
    EXP = mybir.ActivationFunctionType.Exp
    MULT = mybir.AluOpType.mult

    nc = bacc.Bacc("TRN2", target_bir_lowering=False, debug=False,
                   enable_asserts=False, num_devices=NCORES)

    qT = nc.dram_tensor("qT", [D, S], BF16, kind="ExternalInput")
    kT = nc.dram_tensor("kT", [D, S], BF16, kind="ExternalInput")
    vT = nc.dram_tensor("vT", [D, S], BF16, kind="ExternalInput")
    maskP = nc.dram_tensor("maskP", [NQT, 128, KB, QT], U8,
                           kind="ExternalInput")
    wq = nc.dram_tensor("wq", [D, HP * DK], BF16, kind="ExternalInput")
    wk = nc.dram_tensor("wk", [D, HP * DK], BF16, kind="ExternalInput")
    wv = nc.dram_tensor("wv", [D, HP * DK], BF16, kind="ExternalInput")
    wo = nc.dram_tensor("wo", [HP * DK, D], BF16, kind="ExternalInput")
    out = nc.dram_tensor("out", [S, D], F32, kind="ExternalOutput")
    # DRAM bounce buffer for broadcasting softmax reciprocals across partitions
    rcd = nc.dram_tensor("rcd", [NQT * HP, 512], F32, kind="Internal")

    import concourse.bass as bass

    with tile.TileContext(nc) as tc:
        with tc.tile_pool(name="persist", bufs=1) as persist, \
             tc.tile_pool(name="maskp", bufs=2) as maskp, \
             tc.tile_pool(name="masku", bufs=2) as masku, \
             tc.tile_pool(name="pstg", bufs=3) as pstg, \
             tc.tile_pool(name="pxt", bufs=2) as pxt, \
             tc.tile_pool(name="pwrk", bufs=3) as pwrk, \
             tc.tile_pool(name="pex", bufs=6) as pex, \
             tc.tile_pool(name="late", bufs=1) as late, \
             tc.tile_pool(name="psc", bufs=2, space="PSUM") as psc, \
             tc.tile_pool(name="ppv", bufs=2, space="PSUM") as ppv:

            ppv._bctag = "pvb"

            # ---------------- persistent tiles ----------------
            KT = persist.tile([128, S], BF16)        # K^T (2 heads' dk rows)
            QTt = persist.tile([128, S], BF16)       # Q^T
            VA = persist.tile([128, KB, HP * 65], BF16)  # V + ones col/head
            ones_t = persist.tile([128, 64], F32)
            nc.vector.memset(ones_t, 1.0)
            WO128 = persist.tile([128, D], BF16)     # both heads' wo rows
            wk_b = persist.tile([128, 4, 128], BF16)
            wq_b = persist.tile([128, 4, 128], BF16)
            wv_b = persist.tile([128, 4, 128], BF16)

            va_ones = VA.rearrange("p kb (h x) -> p kb h x", x=65)[:, :, :, 64:65]
            nc.gpsimd.memset(va_ones, 1.0)

            kT_src = kT[:, :].rearrange("(c p) s -> p c s", p=128)
            vT_src = vT[:, :].rearrange("(c p) s -> p c s", p=128)
            qT_src = qT[:, :].rearrange("(c p) s -> p c s", p=128)

            maskq = {}
            masku_t = {}

            def get_mq(qt):
                if qt not in maskq:
                    maskq[qt] = maskp.tile([128, KB, QT], BF16, tag="mq",
                                           name=f"mq{qt}")
                return maskq[qt]

            def load_masku(qt, quarter):
                # raw uint8 quarter (half the DMA bytes of the casting path)
                if qt not in masku_t:
                    masku_t[qt] = masku.tile([128, KB, QT], U8, tag="mu",
                                             name=f"mu{qt}")
                nc.gpsimd.dma_start(
                    out=masku_t[qt][:, quarter * 8:(quarter + 1) * 8, :],
                    in_=maskP[qt, :, quarter * 8:(quarter + 1) * 8, :])

            def conv_mask(qt, quarter):
                # u8 -> bf16 on the otherwise-idle GPSIMD engine
                nc.gpsimd.tensor_copy(
                    get_mq(qt)[:, quarter * 8:(quarter + 1) * 8, :],
                    masku_t[qt][:, quarter * 8:(quarter + 1) * 8, :])

            def load_mask(qt, mc0=0, mc1=4):
                # 4 preemptible sub-DMAs so latency-sensitive transfers
                # (rcd bounce) interleave between them
                mq = get_mq(qt)
                for mc in range(4 * mc0, 4 * mc1):
                    nc.gpsimd.dma_start(
                        out=mq[:, mc * 2:(mc + 1) * 2, :],
                        in_=maskP[qt, :, mc * 2:(mc + 1) * 2, :])

            # ---------------- projections (quarter-staged) ----------------
            # All proj psum borrows score-ring tiles ([128, 3, 512] = 3
            # banks), so projections and attention share one PSUM layout.
            def stage(src, qtr, name):
                t = pstg.tile([128, 4, QTR], BF16, tag="stg", name=name)
                nc.gpsimd.dma_start(
                    out=t, in_=src[:, :, qtr * QTR:(qtr + 1) * QTR])
                return t

            def proj_kq(w_b, tin, dst, toff, nst, name):
                # dst[:, toff : toff+nst*512] from one staged quarter
                pt = psc.tile([128, G, QT], F32, tag="sc", name=f"pp{name}")
                for ns in range(nst):
                    for di in range(4):
                        nc.tensor.matmul(
                            pt[:, ns, :], w_b[:, di, :],
                            tin[:, di, ns * 512:(ns + 1) * 512],
                            start=(di == 0), stop=(di == 3))
                nc.vector.tensor_copy(
                    dst[:, toff:toff + nst * 512],
                    pt[:, 0:nst, :].rearrange("p a b -> p (a b)"))

            def proj_v(tin, sc0):
                # 8 VA chunks [128 s-rows, 128 v-cols] from one staged quarter
                pt = psc.tile([128, G, QT], F32, tag="sc", name=f"ppv{sc0}")
                for i in range(8):
                    si = (sc0 + i) % 8
                    for di in range(4):
                        nc.tensor.matmul(
                            pt[:, i // 4, (i % 4) * 128:(i % 4 + 1) * 128],
                            tin[:, di, si * 128:(si + 1) * 128],
                            wv_b[:, di, :],
                            start=(di == 0), stop=(di == 3))
                src = pt.rearrange("p g (qc hx) -> p (g qc) hx", hx=128)
                src = src.rearrange("p c (h x) -> p c h x", x=64)[:, 0:8]
                dst = VA[:, sc0:sc0 + 8, :].rearrange(
                    "p kb (h x) -> p kb h x", x=65)[:, :, :, 0:64]
                nc.vector.tensor_copy(dst, src)

            # prologue DMA order is latency-critical: first-needed first.
            # Only K quarter 0 / Q quarter 0 / mask quarter 0 / V quarter 0
            # are projected before attention emission starts; everything
            # else drains into early attention groups in consumption order.
            kq0 = stage(kT_src, 0, "kq0")
            nc.gpsimd.dma_start(
                out=wk_b, in_=wk[:, :].rearrange("(c p) d -> p c d", p=128))
            qq0 = stage(qT_src, 0, "qq0")
            nc.gpsimd.dma_start(
                out=wq_b, in_=wq[:, :].rearrange("(c p) d -> p c d", p=128))
            load_mask(0, 0, 1)
            vq0 = stage(vT_src, 0, "vq0")
            nc.gpsimd.dma_start(
                out=wv_b, in_=wv[:, :].rearrange("(c p) d -> p c d", p=128))
            proj_kq(wk_b, kq0, KT, 0, 2, "k0")
            kq1 = stage(kT_src, 1, "kq1")
            proj_kq(wq_b, qq0, QTt, 0, 2, "q0")
            load_mask(0, 1, 2)
            vq1 = stage(vT_src, 1, "vq1")
            staged = {("kq", 1): kq1, ("v", 0): vq0, ("v", 1): vq1}

            # drained actions, keyed by (unit, group) emission slots; each
            # runs on the shared psc ring / DMA queues in consumption order
            def a_dma_kq(qtr):
                return lambda: staged.__setitem__(
                    ("kq", qtr), stage(kT_src, qtr, f"kq{qtr}"))

            def a_dma_vq(qtr):
                return lambda: staged.__setitem__(
                    ("v", qtr), stage(vT_src, qtr, f"vq{qtr}"))

            def a_dma_qq(qtr):
                return lambda: staged.__setitem__(
                    ("q", qtr), stage(qT_src, qtr, f"qq{qtr}"))

            def a_k(qtr):
                return lambda: proj_kq(wk_b, staged.pop(("kq", qtr)), KT,
                                       qtr * QTR, 2, f"k{qtr}")

            def a_v(qtr):
                return lambda: proj_v(staged.pop(("v", qtr)), qtr * 8)

            def a_q(qtr):
                return lambda: proj_kq(wq_b, staged.pop(("q", qtr)), QTt,
                                       qtr * QTR, 2, f"q{qtr}")

            def a_m0(quarter):
                return lambda: load_mask(0, quarter, quarter + 1)

            def a_m1(quarter):
                return lambda: load_mask(1, quarter, quarter + 1)

            def a_wo():
                return lambda: nc.gpsimd.dma_start(out=WO128, in_=wo[:, :])

            drain_slots = {
                (0, 0): [a_k(1), a_v(0), a_m0(2)],
                (0, 1): [a_dma_kq(2), a_dma_vq(2)],
                (0, 2): [a_v(1), a_m0(3)],
                (0, 3): [a_k(2)],
                (0, 5): [a_v(2), a_dma_kq(3), a_dma_vq(3)],
                (0, 6): [a_k(3)],
                (0, 8): [a_v(3)],
                (0, 10): [a_wo()],
                (2, 1): [a_dma_qq(1)],
                (2, 5): [a_q(1)],
                (2, 7): [a_dma_qq(2)],
                (3, 5): [a_q(2)],
                (3, 7): [a_dma_qq(3)],
                (4, 5): [a_q(3)],
            }

            def drain_task(ui, gi):
                for a in drain_slots.get((ui, gi), ()):
                    a()

            # ---------------- attention ----------------
            units = [(qt, h) for qt in range(NQT) for h in range(HP)]
            xts = {}

            def emit_norm1(ui, pv):
                # reciprocal of the sums row, bounced through DRAM to
                # broadcast it across partitions 0-63 (SBUF APs cannot
                # have a zero partition stride; DRAM APs can). The SP
                # queue carries only this + the small out-writes, so the
                # latency-sensitive bounce never queues behind bulk DMAs.
                rc = pwrk.tile([128, QT], F32, tag="rc", name=f"rc{ui}")
                nc.vector.reciprocal(rc[64:65, :], pv[64:65, :])
                nc.sync.dma_start(out=rcd[ui:ui + 1, :], in_=rc[64:65, :])
                bcs = pwrk.tile([64, QT], F32, tag="bcs", name=f"bcs{ui}")
                src = rcd[ui:ui + 1, :]
                bsrc = bass.AP(tensor=src.tensor, offset=src.offset,
                               ap=[[0, 64]] + [list(a) for a in src.ap[1:]])
                nc.sync.dma_start(out=bcs, in_=bsrc)
                return bcs

            def emit_norm2(qt, h, pv, bcs):
                # head h lands at partitions h*64 .. h*64+63 (packed xt)
                nc.vector.tensor_tensor(
                    xts[qt][h * 64:(h + 1) * 64, :], pv[0:64, :], bcs, op=MULT)

            groups = []
            kb0 = 0
            while kb0 < KB:
                groups.append((kb0, min(G, KB - kb0)))
                kb0 += G

            def emit_scores(qt, h, kb0, gn):
                pb = h * 64
                sc = psc.tile([128, G, QT], F32, tag="sc")
                for i in range(gn):
                    kb = kb0 + i
                    nc.tensor.matmul(
                        sc[:, i, :],
                        KT[pb:pb + 64, kb * 128:(kb + 1) * 128],
                        QTt[pb:pb + 64, qt * QT:(qt + 1) * QT],
                        start=True, stop=True)
                return sc

            pending_wo = []

            def emit_wo(qt, qc):
                po = ppv.tile([128, D], F32, tag="pvb", name=f"po{qt}_{qc}")
                nc.tensor.matmul(
                    po, xts[qt][:, qc * 128:(qc + 1) * 128], WO128,
                    start=True, stop=True)
                outt = late.tile([128, D], F32, tag="outt", bufs=3)
                nc.vector.tensor_copy(outt, po)
                nc.sync.dma_start(
                    out=out[qt * QT + qc * 128:qt * QT + (qc + 1) * 128, :],
                    in_=outt)

            flat = []
            for ui in range(len(units)):
                qt, h = units[ui]
                for gi, (kb0, gn) in enumerate(groups):
                    flat.append((ui, qt, h, gi, kb0, gn))
            # interleave the two units of qt0 group-by-group: the first
            # query tile's window is input-DMA-bound, and both units consume
            # the same K/V/mask chunks -- alternating them doubles the exp
            # work available per arrived chunk and hides the DMA latency
            ng = len(groups)
            head, rest = flat[:2 * ng], flat[2 * ng:]
            inter = []
            for g in range(ng):
                inter.append(head[g])
                inter.append(head[ng + g])
            flat = inter + rest

            sc_tiles = {}

            def emit_sc(idx):
                _, qt, h, _, kb0, gn = flat[idx]
                sc_tiles[idx] = emit_scores(qt, h, kb0, gn)

            emit_sc(0)
            emit_sc(1)
            pv_of = {}
            pending_norms = []
            for idx, (ui, qt, h, gi, kb0, gn) in enumerate(flat):
                if h == 0 and gi == 0 and qt not in xts:
                    xts[qt] = pxt.tile([128, QT], BF16, tag="xt",
                                       name=f"xt{qt}")
                if h == 0 and gi in (0, 3, 6, 9) and qt + 1 < NQT:
                    load_masku(qt + 1, gi // 3)
                if h == 1 and gi in (1, 4, 7, 10) and qt + 1 < NQT:
                    conv_mask(qt + 1, {1: 0, 4: 1, 7: 2, 10: 3}[gi])
                if gi == 0:
                    pv_of[ui] = ppv.tile([128, QT], F32, tag="pvb",
                                         name=f"pv{ui}")
                pv = pv_of[ui]
                # drains BEFORE the score prefetch: drained projections must
                # precede, in emission order, any consumer of their outputs
                drain_task(ui, gi)
                if idx + 2 < len(flat):
                    emit_sc(idx + 2)
                sc = sc_tiles.pop(idx)
                ex = pex.tile([128, G, QT], BF16, tag="ex")
                nc.scalar.activation(ex[:, 0:gn, :], sc[:, 0:gn, :],
                                     EXP, scale=0.125)
                mk = pex.tile([128, G, QT], BF16, tag="mk")
                nc.vector.tensor_tensor(
                    mk[:, 0:gn, :], ex[:, 0:gn, :],
                    maskq[qt][:, kb0:kb0 + gn, :], op=MULT)
                for i in range(gn):
                    kb = kb0 + i
                    nc.tensor.matmul(
                        pv[0:65, :],
                        VA[:, kb, h * 65:(h + 1) * 65],
                        mk[:, i, :],
                        start=(kb == 0), stop=(kb == KB - 1))
                if gi in (2, 4) and pending_norms:
                    # deferred normalize-multiply of a previous unit (its
                    # pv slot frees here, mid-unit, so the boundary never
                    # serializes on the norm chain)
                    emit_norm2(*pending_norms.pop(0))
                elif gi in (5, 6, 7, 8) and pending_wo:
                    emit_wo(*pending_wo.pop(0))
                if gi == len(groups) - 1:
                    if ui == len(units) - 1:
                        # final unit: fast-path norm via PE broadcast
                        # (score PSUM slots are free at this point)
                        rc = pwrk.tile([128, QT], F32, tag="rc", name="rcF")
                        nc.vector.reciprocal(rc[64:65, :], pv[64:65, :])
                        bct = psc.tile([128, G, QT], F32, tag="sc",
                                       name="bcF")
                        nc.tensor.matmul(bct[0:64, 0, :], ones_t[64:65, :],
                                         rc[64:65, :], start=True, stop=True)
                        bcs = pwrk.tile([64, QT], F32, tag="bcs", name="bcsF")
                        nc.vector.tensor_copy(bcs, bct[0:64, 0, :])
                        emit_norm2(qt, h, pv, bcs)
                        pending_wo.extend(
                            (qt, qc) for qc in range(QT // 128))
                        while pending_wo:
                            emit_wo(*pending_wo.pop(0))
                    else:
                        bcs = emit_norm1(ui, pv)
                        pending_norms.append((qt, h, pv, bcs))
                        if h == HP - 1:
                            pending_wo.extend(
                                (qt, qc) for qc in range(QT // 128))

    nc.compile()
    nc.m = get_hw_module(nc.m)
    return nc


def _get_built():
    global _BUILT
    if _BUILT is None:
        _BUILT = _build()
    return _BUILT


def kernel(q, k, v, mask, w_q, w_k, w_v, w_o):
    import os
    # NTFF tracing needs antenv.axon_hooks, absent in some environments;
    # never let an inherited BASS_TRACE env var route us into that path.
    os.environ.setdefault("BASS_NEVER_TRACE", "1")
    import ml_dtypes
    from concourse.bass_utils import run_bass_kernel_spmd

    bf16 = ml_dtypes.bfloat16

    q = np.asarray(q, dtype=np.float32)
    k = np.asarray(k, dtype=np.float32)
    v = np.asarray(v, dtype=np.float32)
    mask = np.asarray(mask, dtype=np.int32)
    w_q = np.asarray(w_q, dtype=np.float32)
    w_k = np.asarray(w_k, dtype=np.float32)
    w_v = np.asarray(w_v, dtype=np.float32)
    w_o = np.asarray(w_o, dtype=np.float32)

    nc = _get_built()

    qT = [np.ascontiguousarray(q[b].T).astype(bf16) for b in range(B)]
    kT = [np.ascontiguousarray(k[b].T).astype(bf16) for b in range(B)]
    vT = [np.ascontiguousarray(v[b].T).astype(bf16) for b in range(B)]
    # maskP[qt, p, kb, q] = mask[b, qt*512+q, kb*128+p], pre-arranged so
    # each query tile's mask is one contiguous-run DMA
    maskP = [np.ascontiguousarray(
        mask[b].astype(np.uint8).reshape(NQT, QT, KB, 128)
        .transpose(0, 3, 2, 1)) for b in range(B)]

    in_maps = []
    for c in range(NCORES):
        b, hp = divmod(c, 4)
        cs = hp * HP * DK
        ce = cs + HP * DK
        in_maps.append({
            "qT": qT[b], "kT": kT[b], "vT": vT[b], "maskP": maskP[b],
            "wq": np.ascontiguousarray(w_q[:, cs:ce]).astype(bf16),
            "wk": np.ascontiguousarray(w_k[:, cs:ce]).astype(bf16),
            "wv": np.ascontiguousarray(w_v[:, cs:ce]).astype(bf16),
            "wo": np.ascontiguousarray(w_o[cs:ce, :]).astype(bf16),
        })

    global _LAST_IN_MAPS
    _LAST_IN_MAPS = in_maps
    res = run_bass_kernel_spmd(nc, in_maps, list(range(NCORES)))

    # Megatron row-parallel unshard: sum the 4 partial w_o contributions
    full = np.empty((B, S, D), dtype=np.float32)
    for b in range(B):
        acc = np.zeros((S, D), dtype=np.float32)
        for hp in range(4):
            acc += np.asarray(res.results[b * 4 + hp]["out"],
                              dtype=np.float32)
        full[b] = acc
    return full
